# revision 1
# baseline (speedup 1.0000x reference)
"""Trainium2 Bass kernel for Convpass-swintransformer hypernet-mask adapter.

Data-parallel over batch: 8 NeuronCores x 8 samples each. All weights
replicated; x is host-transposed to channel-major so every on-device matmul
consumes natural layouts (no on-device transposes).
"""
import sys

sys.path.insert(0, "/opt/trn_rl_repo")

import numpy as np

import concourse.bass as bass
import concourse.tile as tile
from concourse import bacc, mybir
from concourse.bass_utils import run_bass_kernel_spmd

AF = mybir.ActivationFunctionType
FP32 = mybir.dt.float32

B, L, C = 64, 784, 384
DIM, NM, META = 64, 16, 64
HH, WW = 28, 28
NCORES = 8
S = B // NCORES          # samples per core
KC = C // 128            # 3 contraction chunks for C=384
NPOS = [(0, 512), (512, 272)]   # 784 split at psum-bank boundary
QSCALE = 1.702

_CACHE = {}


def _build_nc():
    nc = bacc.Bacc(None)
    d = nc.declare_dram_parameter
    xt_d = d("xt", [S, KC, 128, L], FP32, isOutput=False)
    wa_d = d("wa", [KC, 128, 128], FP32, isOutput=False)
    ba_d = d("ba", [1, 128], FP32, isOutput=False)
    mw2_d = d("mw2", [META, META], FP32, isOutput=False)
    mb2_d = d("mb2", [1, META], FP32, isOutput=False)
    mtT_d = d("mtT", [META, NM], FP32, isOutput=False)
    upw_d = d("upw", [DIM, C], FP32, isOutput=False)
    upb_d = d("upb", [1, C], FP32, isOutput=False)
    hw_d = d("hw", [128, 9, 2048], FP32, isOutput=False)
    hb_d = d("hb", [64, 576], FP32, isOutput=False)
    out_d = d("out", [S, L, C], FP32, isOutput=True)

    with tile.TileContext(nc) as tc:
        with tc.tile_pool(name="consts", bufs=1) as cp, \
             tc.tile_pool(name="xdp", bufs=S) as xdp, \
             tc.tile_pool(name="cwall", bufs=1) as cwp, \
             tc.tile_pool(name="cws", bufs=S) as cwsp:
            # ---- constants ----
            wa = cp.tile([128, KC, 128], FP32)
            nc.sync.dma_start(out=wa[:], in_=wa_d[:].rearrange("k p m -> p k m"))
            ba = cp.tile([1, 128], FP32)
            nc.sync.dma_start(out=ba[:], in_=ba_d[:])
            mw2 = cp.tile([META, META], FP32)
            nc.sync.dma_start(out=mw2[:], in_=mw2_d[:])
            mb2 = cp.tile([1, META], FP32)
            nc.sync.dma_start(out=mb2[:], in_=mb2_d[:])
            mtT = cp.tile([META, NM], FP32)
            nc.sync.dma_start(out=mtT[:], in_=mtT_d[:])
            upw = cp.tile([DIM, C], FP32)
            nc.sync.dma_start(out=upw[:], in_=upw_d[:])
            upb = cp.tile([1, C], FP32)
            nc.sync.dma_start(out=upb[:], in_=upb_d[:])
            hb = cp.tile([64, 576], FP32)
            nc.sync.dma_start(out=hb[:], in_=hb_d[:])
            ones1 = cp.tile([1, L], FP32)
            nc.vector.memset(ones1[:], 1.0)
            ones16 = cp.tile([NM, 64], FP32)
            nc.vector.memset(ones16[:], 1.0)
            featT2 = cp.tile([128, 32], FP32)
            nc.vector.memset(featT2[:], 0.0)

            xd_tiles = []

            # ================= phase A: meta-net / masks / feat =============
            with tc.tile_pool(name="xtp", bufs=3) as xtp, \
                 tc.tile_pool(name="psA", bufs=2, space="PSUM") as psA, \
                 tc.tile_pool(name="psB", bufs=2, space="PSUM") as psB, \
                 tc.tile_pool(name="sbA", bufs=3) as sbA, \
                 tc.tile_pool(name="smallA", bufs=2) as smA:
                for s in range(S):
                    xt = xtp.tile([128, KC, L], FP32, tag="xt")
                    nc.sync.dma_start(
                        out=xt[:], in_=xt_d[s].rearrange("k p q -> p k q"))
                    psa = psA.tile([128, L], FP32, tag="psa")
                    for n0, nw in NPOS:
                        for k in range(KC):
                            nc.tensor.matmul(
                                psa[:, n0:n0 + nw], lhsT=wa[:, k, :],
                                rhs=xt[:, k, n0:n0 + nw],
                                start=(k == 0), stop=False)
                        nc.tensor.matmul(
                            psa[:, n0:n0 + nw], lhsT=ba[:],
                            rhs=ones1[:, n0:n0 + nw], start=False, stop=True)
                    h = sbA.tile([META, L], FP32, tag="h")
                    nc.scalar.activation(h[:], psa[0:META, :], AF.Relu)
                    xd = xdp.tile([DIM, L], FP32)
                    nc.vector.tensor_copy(xd[:], psa[META:128, :])
                    xd_tiles.append(xd)

                    psp = psB.tile([META, L], FP32, tag="psb")
                    for n0, nw in NPOS:
                        nc.tensor.matmul(psp[:, n0:n0 + nw], lhsT=mw2[:],
                                         rhs=h[:, n0:n0 + nw],
                                         start=True, stop=False)
                        nc.tensor.matmul(psp[:, n0:n0 + nw], lhsT=mb2[:],
                                         rhs=ones1[:, n0:n0 + nw],
                                         start=False, stop=True)
                    prompt = sbA.tile([META, L], FP32, tag="prompt")
                    nc.scalar.activation(prompt[:], psp[:], AF.Copy)

                    psm = psB.tile([NM, L], FP32, tag="psb")
                    for n0, nw in NPOS:
                        nc.tensor.matmul(psm[:, n0:n0 + nw], lhsT=mtT[:],
                                         rhs=prompt[:, n0:n0 + nw],
                                         start=True, stop=True)
                    expt = sbA.tile([NM, L], FP32, tag="expt")
                    zsum = smA.tile([NM, 1], FP32, tag="z")
                    nc.scalar.activation(expt[:], psm[:], AF.Exp,
                                         accum_out=zsum[:])
                    invz = smA.tile([NM, 1], FP32, tag="iz")
                    nc.vector.reciprocal(invz[:], zsum[:])
                    expn = sbA.tile([NM, L], FP32, tag="expn")
                    nc.vector.tensor_scalar_mul(expn[:], expt[:], invz[:])

                    pss = psB.tile([64, L], FP32, tag="psb")
                    for n0, nw in NPOS:
                        nc.tensor.matmul(pss[:, n0:n0 + nw], lhsT=ones16[:],
                                         rhs=expn[:, n0:n0 + nw],
                                         start=True, stop=True)
                    ftmp = sbA.tile([64, L], FP32, tag="ftmp")
                    nc.vector.tensor_mul(ftmp[:], pss[:], prompt[:])
                    nc.vector.reduce_sum(featT2[0:64, s:s + 1], ftmp[:],
                                         axis=mybir.AxisListType.X)

            # duplicate feats into upper partition half for split-K hypernet
            nc.sync.dma_start(out=featT2[64:128, :], in_=featT2[0:64, :])

            # ================= phase H: hypernet conv weights ===============
            # Column block j8 of a sample's conv weight lands in psum group
            # g=j8//3 at base partition 32*(j8%3)+s (psum matmul base must be
            # in {0,32,64}).  Whole-tile drains; junk partitions never read.
            GROUPS = [(0, 3), (3, 3), (6, 2)]   # (first j8, blocks) per group
            cwalls = [cwp.tile([32 * nb, 9 * 512], FP32,
                               name=f"cwall{g}", tag=f"cwall{g}")
                      for g, (_, nb) in enumerate(GROUPS)]
            with tc.tile_pool(name="hwp", bufs=2) as hwp, \
                 tc.tile_pool(name="psH", bufs=6, space="PSUM") as psH:
                for n9 in range(9):
                    hwc = hwp.tile([128, 2048], FP32, tag="hw")
                    nc.scalar.dma_start(out=hwc[:], in_=hw_d[:, n9, :])
                    for g, (j8_0, nb) in enumerate(GROUPS):
                        psh = psH.tile([32 * nb, 512], FP32, tag="psh")
                        for slot in range(nb):
                            j8 = j8_0 + slot
                            h2, j4 = divmod(j8, 4)
                            nc.tensor.matmul(
                                psh[32 * slot:32 * slot + 32, :],
                                lhsT=featT2[h2 * 64:(h2 + 1) * 64, :],
                                rhs=hwc[h2 * 64:(h2 + 1) * 64,
                                        j4 * 512:(j4 + 1) * 512],
                                start=True, stop=True)
                        if n9 % 2 == 0:
                            nc.vector.tensor_copy(
                                cwalls[g][:, n9 * 512:(n9 + 1) * 512], psh[:])
                        else:
                            nc.scalar.activation(
                                cwalls[g][:, n9 * 512:(n9 + 1) * 512], psh[:],
                                AF.Copy)

            cw_tiles = []
            for s in range(S):
                cw = cwsp.tile([64, 576], FP32)
                for g, (j8_0, nb) in enumerate(GROUPS):
                    for slot in range(nb):
                        j8 = j8_0 + slot
                        p0 = 32 * slot + s
                        nc.sync.dma_start(
                            out=cw[8 * j8:8 * (j8 + 1), :],
                            in_=cwalls[g][p0:p0 + 1].rearrange(
                                "p (a b) -> p a b", a=8))
                nc.vector.tensor_add(cw[:], cw[:], hb[:])
                cw_tiles.append(cw)

            # ================= phase B: adapter conv + up ===================
            with tc.tile_pool(name="padp", bufs=2) as padp, \
                 tc.tile_pool(name="sgp", bufs=2) as sgp, \
                 tc.tile_pool(name="yap", bufs=2) as yap, \
                 tc.tile_pool(name="outp", bufs=3) as outp, \
                 tc.tile_pool(name="psC0", bufs=2, space="PSUM") as psC0, \
                 tc.tile_pool(name="psC1", bufs=2, space="PSUM") as psC1, \
                 tc.tile_pool(name="psU", bufs=4, space="PSUM") as psU:
                for s in range(S):
                    xd = xd_tiles[s]
                    pad = padp.tile([64, 900], FP32, tag="pad")
                    nc.gpsimd.memset(pad[:], 0.0)
                    sg = sgp.tile([DIM, L], FP32, tag="sg")
                    nc.scalar.activation(sg[:], xd[:], AF.Sigmoid, scale=QSCALE)
                    pad3 = pad.rearrange("p (r c) -> p r c", r=30)
                    nc.vector.tensor_mul(
                        pad3[:, 1:29, 1:29],
                        sg.rearrange("p (a b) -> p a b", a=28)[:],
                        xd.rearrange("p (a b) -> p a b", a=28)[:])

                    ps0 = psC0.tile([64, 448], FP32, tag="c0")
                    ps1 = psC1.tile([64, 336], FP32, tag="c1")
                    cwv = cw_tiles[s].rearrange("p (o k) -> p k o", k=9)
                    for k9 in range(9):
                        ky, kx = divmod(k9, 3)
                        lw = cwv[:, k9, :]
                        nc.tensor.matmul(
                            ps0[:], lhsT=lw,
                            rhs=pad3[:, ky:ky + 16, kx:kx + 28],
                            start=(k9 == 0), stop=(k9 == 8))
                        nc.tensor.matmul(
                            ps1[:], lhsT=lw,
                            rhs=pad3[:, ky + 16:ky + 28, kx:kx + 28],
                            start=(k9 == 0), stop=(k9 == 8))

                    ya = yap.tile([DIM, L], FP32, tag="ya")
                    ys0 = sgp.tile([64, 448], FP32, tag="ys0")
                    nc.scalar.activation(ys0[:], ps0[:], AF.Sigmoid, scale=QSCALE)
                    nc.vector.tensor_mul(ya[:, 0:448], ys0[:], ps0[:])
                    ys1 = sgp.tile([64, 336], FP32, tag="ys1")
                    nc.scalar.activation(ys1[:], ps1[:], AF.Sigmoid, scale=QSCALE)
                    nc.vector.tensor_mul(ya[:, 448:784], ys1[:], ps1[:])

                    outt = outp.tile([112, 7, C], FP32, tag="outt")
                    for j in range(7):
                        psu = psU.tile([112, C], FP32, tag="psu")
                        nc.tensor.matmul(psu[:], lhsT=ya[:, j * 112:(j + 1) * 112],
                                         rhs=upw[:], start=True, stop=False)
                        nc.tensor.matmul(psu[:], lhsT=ones1[:, 0:112],
                                         rhs=upb[:], start=False, stop=True)
                        if j % 2 == 0:
                            nc.scalar.activation(outt[:, j, :], psu[:], AF.Copy)
                        else:
                            nc.vector.tensor_copy(outt[:, j, :], psu[:])
                    nc.sync.dma_start(
                        out=out_d[s].rearrange("(j p) c -> p j c", p=112),
                        in_=outt[:])
    nc.finalize()
    return nc


def _prep(x, meta_w1, meta_b1, meta_w2, meta_b2, mask_token,
          hyper_w, hyper_b, down_w, down_b, up_w, up_b):
    f = lambda a: np.ascontiguousarray(np.asarray(a, dtype=np.float32))
    x = f(x)
    xt = np.ascontiguousarray(x.reshape(B, L, C).transpose(0, 2, 1))  # [B,C,L]
    xt = xt.reshape(B, KC, 128, L)

    wA = np.concatenate([f(meta_w1), f(down_w)], axis=1)        # [384,128]
    wa = np.ascontiguousarray(wA.reshape(KC, 128, 128))
    ba = np.concatenate([f(meta_b1), f(down_b)])[None, :]       # [1,128]
    mtT = np.ascontiguousarray(f(mask_token).T)                 # [64,16]

    hw5 = f(hyper_w).reshape(META, DIM, DIM, 3, 3)
    hwr = np.ascontiguousarray(hw5.transpose(0, 2, 1, 3, 4)).reshape(
        META, 8, 8, 576).reshape(META, 8, 9, 512)               # [n,j8,n9,c]
    top = hwr[:, 0:4].transpose(0, 2, 1, 3)                     # [64,9,4,512]
    bot = hwr[:, 4:8].transpose(0, 2, 1, 3)
    hw128 = np.ascontiguousarray(
        np.concatenate([top, bot], axis=0)).reshape(128, 9, 2048)

    hb5 = f(hyper_b).reshape(DIM, DIM, 3, 3)
    hbcw = np.ascontiguousarray(hb5.transpose(1, 0, 2, 3)).reshape(64, 576)

    consts = {
        "wa": wa, "ba": np.ascontiguousarray(ba),
        "mw2": f(meta_w2), "mb2": f(meta_b2)[None, :],
        "mtT": mtT, "upw": f(up_w), "upb": f(up_b)[None, :],
        "hw": hw128, "hb": hbcw,
    }
    in_maps = []
    for c in range(NCORES):
        m = dict(consts)
        m["xt"] = np.ascontiguousarray(xt[c * S:(c + 1) * S])
        in_maps.append(m)
    return in_maps


def _run(in_maps, **kw):
    if "nc" not in _CACHE:
        _CACHE["nc"] = _build_nc()
    return run_bass_kernel_spmd(_CACHE["nc"], in_maps, list(range(NCORES)), **kw)


def kernel(x, meta_w1, meta_b1, meta_w2, meta_b2, mask_token,
           hyper_w, hyper_b, down_w, down_b, up_w, up_b, H, W):
    assert int(H) == HH and int(W) == WW
    in_maps = _prep(x, meta_w1, meta_b1, meta_w2, meta_b2, mask_token,
                    hyper_w, hyper_b, down_w, down_b, up_w, up_b)
    res = _run(in_maps)
    out = np.concatenate([res.results[c]["out"] for c in range(NCORES)], axis=0)
    return out.reshape(B, L, C).astype(np.float32)



# revision 15
# speedup vs baseline: 3.2023x; 3.2023x over previous
"""Trainium2 Bass kernel for Convpass-swintransformer hypernet-mask adapter.

Data-parallel over batch: 8 NeuronCores x 8 samples each, all weights
replicated. All matmuls run in bf16 (fp32 PSUM accumulation); samples are
processed in pairs with block-diagonal weights so the 64-channel ops fill the
128-wide PE array. Biases are folded into activation/vector drains (or, for
the hypernet, a ones-row on the feat tile) instead of PE bias matmuls.
"""
import sys

sys.path.insert(0, "/opt/trn_rl_repo")

import numpy as np

import concourse.bass as bass
import concourse.tile as tile
from concourse import bacc, mybir
from concourse.bass_utils import run_bass_kernel_spmd

AF = mybir.ActivationFunctionType
ALU = mybir.AluOpType
FP32 = mybir.dt.float32
BF16 = mybir.dt.bfloat16
BF16_NP = mybir.dt.np(BF16)

B, L, C = 64, 784, 384
DIM, NM, META = 64, 16, 64
HH, WW = 28, 28
NCORES = 8
S = B // NCORES          # samples per core
NPAIR = S // 2
KC = C // 128            # 3 contraction chunks for C=384
NPOS = [(0, 512), (512, 272)]   # 784 split at psum-bank boundary
GROUPS = [(0, 3), (3, 3), (6, 2)]   # (first j8, n slots) per hypernet group

_CACHE = {}


def _build_nc():
    nc = bacc.Bacc(None)
    d = nc.declare_dram_parameter
    xt_d = d("xt", [NPAIR, 2, KC, 128, L], BF16, isOutput=False)
    cb_d = d("cb", [128, 2 * 384 + 128 + 32 + 128 + 384], BF16, isOutput=False)
    fb_d = d("fb", [128, 3], FP32, isOutput=False)
    ub_d = d("ub", [128, 3], FP32, isOutput=False)
    hw_d = d("hw", [65, 2, 9, 2048], BF16, isOutput=False)
    out_d = d("out", [NPAIR, 2, KC, 128, L], BF16, isOutput=True)

    with tile.TileContext(nc) as tc:
        with tc.tile_pool(name="consts", bufs=1) as cp, \
             tc.tile_pool(name="xdp", bufs=NPAIR) as xdp, \
             tc.tile_pool(name="padp", bufs=NPAIR) as padp, \
             tc.tile_pool(name="cwp", bufs=NPAIR) as cwpp, \
             tc.tile_pool(name="cwall", bufs=1) as cwp:
            # ---- constants (two blobs) ----
            cb = cp.tile([128, 2 * 384 + 128 + 32 + 128 + 384], BF16)
            nc.sync.dma_start(out=cb[:], in_=cb_d[:])
            fb = cp.tile([128, 3], FP32)
            nc.sync.dma_start(out=fb[:], in_=fb_d[:])
            upb3 = cp.tile([128, 3], FP32)
            nc.sync.dma_start(out=upb3[:], in_=ub_d[:])
            wab = [cb[:, 0:384].rearrange("p (k m) -> p k m", k=KC),
                   cb[:, 384:768].rearrange("p (k m) -> p k m", k=KC)]
            mw2d = cb[:, 768:896]
            mtT2 = cb[:, 896:928]            # [128 n-pair, 32 m-pair]
            ones32 = cb[0:32, 928:1056]
            upw = cb[:, 1056:1440]   # upw duplicated on both partition halves
            b_a = [fb[:, 0:1], fb[:, 1:2]]   # per-sample-parity psa bias
            mb2p = fb[:, 2:3]

            feat65 = cp.tile([65, 32], FP32)
            nc.vector.memset(feat65[:], 0.0)
            nc.vector.memset(feat65[64:65, :], 1.0)

            xd_tiles = []
            pad_tiles = []
            cw_tiles = []
            for pr in range(NPAIR):
                xd = xdp.tile([128, L], BF16)
                xd_tiles.append(xd)
                pad = padp.tile([128, 900], BF16)
                nc.gpsimd.memset(pad[:], 0.0)
                pad_tiles.append(pad)
                cw = cwpp.tile([128, 9, 2, 64], BF16)
                nc.gpsimd.memset(cw[:], 0.0)
                cw_tiles.append(cw)

            # ================= phase A: meta-net / masks / feat =============
            with tc.tile_pool(name="xtp", bufs=2) as xtp, \
                 tc.tile_pool(name="psA", bufs=2, space="PSUM") as psA, \
                 tc.tile_pool(name="psB", bufs=2, space="PSUM") as psB, \
                 tc.tile_pool(name="sbA", bufs=2) as sbA, \
                 tc.tile_pool(name="smallA", bufs=2) as smA:
                for pr in range(NPAIR):
                    xt = xtp.tile([128, 2, KC, L], BF16, tag="xt")
                    nc.sync.dma_start(
                        out=xt[:], in_=xt_d[pr].rearrange("s k p q -> p s k q"))
                    hpair = sbA.tile([128, L], BF16, tag="h")
                    xd = xd_tiles[pr]
                    for h in range(2):
                        psa = psA.tile([128, L], FP32, tag="psa")
                        for n0, nw in NPOS:
                            for k in range(KC):
                                nc.tensor.matmul(
                                    psa[:, n0:n0 + nw], lhsT=wab[h][:, k, :],
                                    rhs=xt[:, h, k, n0:n0 + nw],
                                    start=(k == 0), stop=(k == KC - 1))
                        # even parity: h rows 0:64, xd rows 64:128 of psa
                        # odd parity: swapped (host swaps weight columns)
                        nc.scalar.activation(
                            hpair[64 * h:64 * h + 64, :], psa[64 * h:64 * h + 64, :],
                            AF.Relu, bias=b_a[h][64 * h:64 * h + 64])
                        nc.vector.tensor_scalar(
                            xd[64 * h:64 * h + 64, :], psa[64 - 64 * h:128 - 64 * h, :],
                            b_a[h][64 - 64 * h:128 - 64 * h], None, ALU.add)

                    psp = psB.tile([128, L], FP32, tag="psb")
                    for n0, nw in NPOS:
                        nc.tensor.matmul(psp[:, n0:n0 + nw], lhsT=mw2d,
                                         rhs=hpair[:, n0:n0 + nw],
                                         start=True, stop=True)
                    prompt = sbA.tile([128, L], BF16, tag="prompt")
                    nc.vector.tensor_scalar(prompt[:], psp[:], mb2p, None, ALU.add)

                    psm = psB.tile([32, L], FP32, tag="psb")
                    for n0, nw in NPOS:
                        nc.tensor.matmul(psm[:, n0:n0 + nw], lhsT=mtT2,
                                         rhs=prompt[:, n0:n0 + nw],
                                         start=True, stop=True)
                    expt = sbA.tile([32, L], BF16, tag="expt")
                    zsum = smA.tile([32, 1], FP32, tag="z")
                    nc.scalar.activation(expt[:], psm[:], AF.Exp,
                                         accum_out=zsum[:])
                    invz = smA.tile([32, 1], FP32, tag="iz")
                    nc.vector.reciprocal(invz[:], zsum[:])
                    expn = sbA.tile([32, L], BF16, tag="expn")
                    nc.gpsimd.tensor_scalar_mul(expn[:], expt[:], invz[:])

                    pss = psB.tile([128, L], FP32, tag="psb")
                    for n0, nw in NPOS:
                        nc.tensor.matmul(pss[:, n0:n0 + nw], lhsT=ones32,
                                         rhs=expn[:, n0:n0 + nw],
                                         start=True, stop=True)
                    ftmp = sbA.tile([128, L], BF16, tag="ftmp")
                    nc.vector.tensor_mul(ftmp[:], pss[:], prompt[:])
                    nc.vector.reduce_sum(feat65[0:64, 2 * pr:2 * pr + 1],
                                         ftmp[0:64, :],
                                         axis=mybir.AxisListType.X)
                    nc.vector.reduce_sum(feat65[0:64, 2 * pr + 1:2 * pr + 2],
                                         ftmp[64:128, :],
                                         axis=mybir.AxisListType.X)

            # ================= phase H: hypernet conv weights ===============
            cwalls = [cwp.tile([32 * nb, 9 * 512], BF16,
                               name=f"cwall{g}", tag=f"cwall{g}")
                      for g, (_, nb) in enumerate(GROUPS)]
            feat_bf = cp.tile([65, 32], BF16)
            nc.vector.tensor_copy(feat_bf[:], feat65[:])
            with tc.tile_pool(name="hwp", bufs=2) as hwp, \
                 tc.tile_pool(name="psH", bufs=6, space="PSUM") as psH:
                rot = 0
                for n9 in range(9):
                    hwc = hwp.tile([65, 2, 2048], BF16, tag="hw")
                    nc.sync.dma_start(out=hwc[:], in_=hw_d[:, :, n9, :])
                    for g, (j8_0, nb) in enumerate(GROUPS):
                        psh = psH.tile([32 * nb, 512], FP32, tag="psh")
                        for a in range(nb):
                            h2, j4 = divmod(j8_0 + a, 4)
                            nc.tensor.matmul(
                                psh[32 * a:32 * a + 32, :],
                                lhsT=feat_bf[:, :],
                                rhs=hwc[:, h2, j4 * 512:(j4 + 1) * 512],
                                start=True, stop=True)
                        dst = cwalls[g][:, n9 * 512:(n9 + 1) * 512]
                        if rot == 0:
                            nc.vector.tensor_copy(dst, psh[:])
                        else:
                            nc.scalar.activation(dst, psh[:], AF.Copy)
                        rot = (rot + 1) % 2

            for s in range(S):
                pr, h = divmod(s, 2)
                for g, (j8_0, nb) in enumerate(GROUPS):
                    nc.sync.dma_start(
                        out=cw_tiles[pr][64 * h + 8 * j8_0:
                                         64 * h + 8 * (j8_0 + nb), :, h, :],
                        in_=cwalls[g][s:32 * nb:32].rearrange(
                            "p (i k o) -> p i k o", i=8, k=9))

            # ================= phase B: adapter conv + up ===================
            with tc.tile_pool(name="yap", bufs=2) as yap, \
                 tc.tile_pool(name="outp", bufs=2) as outp, \
                 tc.tile_pool(name="psC0", bufs=2, space="PSUM") as psC0, \
                 tc.tile_pool(name="psC1", bufs=2, space="PSUM") as psC1, \
                 tc.tile_pool(name="psU", bufs=4, space="PSUM") as psU:
                rot = 0
                for pr in range(NPAIR):
                    pad = pad_tiles[pr]
                    pad3 = pad.rearrange("p (r c) -> p r c", r=30)
                    nc.scalar.activation(
                        pad3[:, 1:29, 1:29],
                        xd_tiles[pr].rearrange("p (a b) -> p a b", a=28)[:],
                        AF.Gelu_apprx_sigmoid)

                    ps0 = psC0.tile([128, 448], FP32, tag="c0")
                    ps1 = psC1.tile([128, 336], FP32, tag="c1")
                    for k9 in range(9):
                        ky, kx = divmod(k9, 3)
                        lw = cw_tiles[pr][:, k9, :, :]
                        nc.tensor.matmul(
                            ps0[:], lhsT=lw,
                            rhs=pad3[:, ky:ky + 16, kx:kx + 28],
                            start=(k9 == 0), stop=(k9 == 8))
                        nc.tensor.matmul(
                            ps1[:], lhsT=lw,
                            rhs=pad3[:, ky + 16:ky + 28, kx:kx + 28],
                            start=(k9 == 0), stop=(k9 == 8))

                    ya = yap.tile([128, L], BF16, tag="ya")
                    nc.scalar.activation(ya[:, 0:448], ps0[:],
                                         AF.Gelu_apprx_sigmoid)
                    nc.scalar.activation(ya[:, 448:784], ps1[:],
                                         AF.Gelu_apprx_sigmoid)

                    outt = outp.tile([128, 2, KC, L], BF16, tag="outt")
                    for h in range(2):
                        for j3 in range(KC):
                            for n0, nw in NPOS:
                                psu = psU.tile([128, nw], FP32, tag="psu")
                                nc.tensor.matmul(
                                    psu[:],
                                    lhsT=upw[64 * h:64 * h + 64,
                                             128 * j3:128 * (j3 + 1)],
                                    rhs=ya[64 * h:64 * h + 64, n0:n0 + nw],
                                    start=True, stop=True)
                                dst = outt[:, h, j3, n0:n0 + nw]
                                if rot == 0:
                                    nc.vector.tensor_scalar(
                                        dst, psu[:], upb3[:, j3:j3 + 1],
                                        None, ALU.add)
                                else:
                                    nc.scalar.activation(
                                        dst, psu[:], AF.Identity,
                                        bias=upb3[:, j3:j3 + 1])
                                rot = (rot + 1) % 2
                    nc.sync.dma_start(
                        out=out_d[pr].rearrange("s k p q -> p s k q"),
                        in_=outt[:])
    nc.finalize()
    return nc


def _prep(x, meta_w1, meta_b1, meta_w2, meta_b2, mask_token,
          hyper_w, hyper_b, down_w, down_b, up_w, up_b):
    f = lambda a: np.ascontiguousarray(np.asarray(a, dtype=np.float32))
    bf = lambda a: np.ascontiguousarray(np.asarray(a).astype(BF16_NP))
    x = f(x)
    xt = x.reshape(B, L, C).transpose(0, 2, 1)            # [B, C, L]
    xt = bf(xt).reshape(B, KC, 128, L).reshape(NCORES, NPAIR, 2, KC, 128, L)

    # psa weights: even parity [meta_w1 | down_w], odd parity swapped
    wA0 = np.concatenate([f(meta_w1), f(down_w)], axis=1)   # [384, 128]
    wA1 = np.concatenate([f(down_w), f(meta_w1)], axis=1)
    wab = np.stack([wA0, wA1]).reshape(2, KC, 128, 128).transpose(
        0, 2, 1, 3).reshape(2, 128, 384)                    # [par, p, (k m)]

    mw2d = np.zeros((128, 128), np.float32)
    mw2d[0:64, 0:64] = f(meta_w2)
    mw2d[64:128, 64:128] = f(meta_w2)
    mtT2s = np.zeros((32, 128), np.float32)     # stored transposed: [m, n]
    mtT2s[0:16, 0:64] = f(mask_token)
    mtT2s[16:32, 64:128] = f(mask_token)
    ones32 = np.zeros((32, 128), np.float32)
    ones32[0:16, 0:64] = 1.0
    ones32[16:32, 64:128] = 1.0
    upw = f(up_w)                                # [64, 384]

    cbw = 2 * 384 + 128 + 32 + 128 + 384
    cb = np.zeros((128, cbw), np.float32)
    cb[:, 0:384] = wab[0]
    cb[:, 384:768] = wab[1]
    cb[:, 768:896] = mw2d
    cb[:, 896:928] = mtT2s.T                    # [128 n-pair, 32 m-pair]
    cb[0:32, 928:1056] = ones32
    cb[0:64, 1056:1440] = upw
    cb[64:128, 1056:1440] = upw                 # dup for base-partition match
    cb = bf(cb)

    fbm = np.zeros((128, 3), np.float32)
    fbm[0:64, 0] = f(meta_b1)
    fbm[64:128, 0] = f(down_b)
    fbm[0:64, 1] = f(down_b)
    fbm[64:128, 1] = f(meta_b1)
    fbm[0:64, 2] = f(meta_b2)
    fbm[64:128, 2] = f(meta_b2)

    upb3 = f(up_b).reshape(KC, 128).T            # [128, 3]

    # hypernet weights: columns packed (j8, i8, ky, kx, o); ones-row = hyper_b
    hw5 = f(hyper_w).reshape(META, DIM, DIM, 3, 3)       # [n, o, i, ky, kx]
    hwc = hw5.transpose(0, 2, 3, 4, 1).reshape(META, 8, 4608)  # [n, j8, (i8 k o)]
    hwc = hwc.reshape(META, 2, 4, 9, 512).transpose(0, 1, 3, 2, 4).reshape(
        META, 2, 9, 2048)
    hb5 = f(hyper_b).reshape(DIM, DIM, 3, 3)             # [o, i, ky, kx]
    hbc = hb5.transpose(1, 2, 3, 0).reshape(8, 4608)     # [j8, (i8 k o)]
    hbc = hbc.reshape(2, 4, 9, 512).transpose(0, 2, 1, 3).reshape(2, 9, 2048)
    hwe = np.concatenate([hwc, hbc[None]], axis=0)       # [65, 2, 9, 2048]
    hwe = bf(hwe)

    consts = {"cb": cb, "fb": fbm, "hw": hwe, "ub": np.ascontiguousarray(upb3)}
    in_maps = []
    for c in range(NCORES):
        m = dict(consts)
        m["xt"] = np.ascontiguousarray(xt[c])
        in_maps.append(m)
    return in_maps


def _run(in_maps, **kw):
    if "nc" not in _CACHE:
        _CACHE["nc"] = _build_nc()
    return run_bass_kernel_spmd(_CACHE["nc"], in_maps, list(range(NCORES)), **kw)


def kernel(x, meta_w1, meta_b1, meta_w2, meta_b2, mask_token,
           hyper_w, hyper_b, down_w, down_b, up_w, up_b, H, W):
    assert int(H) == HH and int(W) == WW
    in_maps = _prep(x, meta_w1, meta_b1, meta_w2, meta_b2, mask_token,
                    hyper_w, hyper_b, down_w, down_b, up_w, up_b)
    res = _run(in_maps)
    outs = []
    for c in range(NCORES):
        o = np.asarray(res.results[c]["out"]).astype(np.float32)
        # [NPAIR, 2, KC, 128, L] -> [S, C, L] -> [S, L, C]
        o = o.reshape(S, C, L).transpose(0, 2, 1)
        outs.append(o)
    out = np.concatenate(outs, axis=0)
    return np.ascontiguousarray(out.reshape(B, L, C)).astype(np.float32)


# revision 45
# speedup vs baseline: 4.2611x; 1.3307x over previous
"""Trainium2 Bass kernel for Convpass-swintransformer hypernet-mask adapter.

Data-parallel over batch: 8 NeuronCores x 8 samples each, all weights
replicated. All matmuls run in bf16 (fp32 PSUM accumulation); samples are
processed in pairs with block-diagonal weights so the 64-channel ops fill the
128-wide PE array. Biases are folded into activation/vector drains (or, for
the hypernet, a ones-row on the feat tile) instead of PE bias matmuls.
QuickGELU is a single Gelu_apprx_sigmoid activation; phase-B gelu ops are
gated behind phase A so the ACT function-table isn't thrashed against Exp.
"""
import sys

sys.path.insert(0, "/opt/trn_rl_repo")

import numpy as np

import concourse.bass as bass
import concourse.tile as tile
from concourse import bacc, mybir
from concourse.bass_utils import run_bass_kernel_spmd

AF = mybir.ActivationFunctionType
ALU = mybir.AluOpType
FP32 = mybir.dt.float32
BF16 = mybir.dt.bfloat16
BF16_NP = mybir.dt.np(BF16)

B, L, C = 64, 784, 384
DIM, NM, META = 64, 16, 64
HH, WW = 28, 28
NCORES = 8
S = B // NCORES          # samples per core
NPAIR = S // 2
KC = C // 128            # 3 contraction chunks for C=384
NPOS = [(0, 512), (512, 272)]   # 784 split at psum-bank boundary
NPOSB = [(0, 448), (448, 336)]  # 784 split matching the conv-psum halves
GROUPS = [(0, 3), (3, 3), (6, 2)]   # (first j8, n slots) per hypernet group

_CACHE = {}


def _build_nc():
    nc = bacc.Bacc(None)
    d = nc.declare_dram_parameter
    xt_d = d("xt", [NPAIR, 2, KC, 128, L], BF16, isOutput=False)
    cb_d = d("cb", [128, 2 * 384 + 128 + 32 + 128 + 384], BF16, isOutput=False)
    fb_d = d("fb", [128, 3], FP32, isOutput=False)
    ub_d = d("ub", [128, 3], FP32, isOutput=False)
    hw_d = d("hw", [65, 2, 9, 2048], BF16, isOutput=False)
    out_d = d("out", [S, KC, 128, L], BF16, isOutput=True)

    with tile.TileContext(nc) as tc:
        with tc.tile_pool(name="consts", bufs=1) as cp, \
             tc.tile_pool(name="xdp", bufs=NPAIR) as xdp, \
             tc.tile_pool(name="padp", bufs=NPAIR) as padp, \
             tc.tile_pool(name="cwp", bufs=NPAIR) as cwpp, \
             tc.tile_pool(name="cwall", bufs=1) as cwp, \
             tc.tile_pool(name="hwp", bufs=9) as hwp:
            # ---- constants (two blobs) ----
            cb = cp.tile([128, 2 * 384 + 128 + 32 + 128 + 384], BF16)
            nc.sync.dma_start(out=cb[:], in_=cb_d[:])
            fb = cp.tile([128, 3], FP32)
            upb3 = cp.tile([128, 3], FP32)
            wab = [cb[:, 0:384].rearrange("p (k m) -> p k m", k=KC),
                   cb[:, 384:768].rearrange("p (k m) -> p k m", k=KC)]
            mw2d = cb[:, 768:896]
            mtT2 = cb[:, 896:928]            # [128 n-pair, 32 m-pair]
            ones32 = cb[0:32, 928:1056]
            upw = cb[:, 1056:1440]           # upw dup'd on both halves
            b_a = [fb[:, 0:1], fb[:, 1:2]]   # per-sample-parity psa bias
            mb2p = fb[:, 2:3]

            feat_bf = cp.tile([65, 32], BF16)
            nc.vector.memset(feat_bf[:], 0.0)
            nc.vector.memset(feat_bf[64:65, :], 1.0)
            featP = cp.tile([128, NPAIR], FP32)
            featQ = cp.tile([128, NPAIR], FP32)
            featPQ = [featP, featQ]
            zgate = cp.tile([128, 1], FP32)

            xd_tiles, pad_tiles, cw_tiles = [], [], []
            for pr in range(NPAIR):
                xd = xdp.tile([128, L], BF16)
                xd_tiles.append(xd)
                pad = padp.tile([128, 900], BF16)
                nc.gpsimd.memset(pad[:], 0.0)
                pad_tiles.append(pad)
                cw = cwpp.tile([128, 9, 2, 64], BF16)
                nc.gpsimd.memset(cw[:], 0.0)
                cw_tiles.append(cw)

            # ================= phase A: meta-net / masks / feat =============
            # Software-pipelined: pair p+1's psa matmuls are interleaved
            # between pair p's small matmuls so the PE never drains, and the
            # ACT/DVE queues are emitted critical-path-first.
            with tc.tile_pool(name="xtp", bufs=4) as xtp, \
                 tc.tile_pool(name="psA", bufs=2, space="PSUM") as psA, \
                 tc.tile_pool(name="psB", bufs=4, space="PSUM") as psB, \
                 tc.tile_pool(name="sbA", bufs=2) as sbA, \
                 tc.tile_pool(name="smallA", bufs=2) as smA:
                xts = [None] * NPAIR
                hpairs = [None] * NPAIR
                prompts = [None] * NPAIR

                def emit_xt_dma(pr):
                    xt = xtp.tile([128, 2, KC, L], BF16, name=f"xt{pr}", tag="xt")
                    xts[pr] = xt
                    if pr == 0:
                        for k in range(KC):     # finest grain: start compute
                            nc.sync.dma_start(   # after 1/6 of the data
                                out=xt[:, 0, k, :],
                                in_=xt_d[0, 0, k].rearrange("p q -> p q"))
                        nc.sync.dma_start(
                            out=xt[:, 1, :, :],
                            in_=xt_d[0, 1].rearrange("k p q -> p k q"))
                        nc.sync.dma_start(out=fb[:], in_=fb_d[:])
                        nc.sync.dma_start(out=upb3[:], in_=ub_d[:])
                    else:
                        nc.sync.dma_start(
                            out=xt[:], in_=xt_d[pr].rearrange(
                                "s k p q -> p s k q"))

                def emit_psa(pr, h):
                    psa = psA.tile([128, L], FP32, tag="psa")
                    for n0, nw in NPOS:
                        for k in range(KC):
                            nc.tensor.matmul(
                                psa[:, n0:n0 + nw], lhsT=wab[h][:, k, :],
                                rhs=xts[pr][:, h, k, n0:n0 + nw],
                                start=(k == 0), stop=(k == KC - 1))
                    return psa

                def emit_relu(pr, h, psa):
                    # even parity: h rows 0:64, xd rows 64:128 of psa; odd
                    # parity swapped (host swaps weight columns) so the ACT
                    # relu never shifts partitions
                    nc.scalar.activation(
                        hpairs[pr][64 * h:64 * h + 64, :],
                        psa[64 * h:64 * h + 64, :],
                        AF.Relu, bias=b_a[h][64 * h:64 * h + 64])

                def emit_xd(pr, h, psa):
                    nc.vector.tensor_scalar(
                        xd_tiles[pr][64 * h:64 * h + 64, :],
                        psa[64 - 64 * h:128 - 64 * h, :],
                        b_a[h][64 - 64 * h:128 - 64 * h], None, ALU.add)

                # prologue: pair 0 psa + drains
                emit_xt_dma(0)
                hpairs[0] = sbA.tile([128, L], BF16, name="hp0", tag="h")
                for h in range(2):
                    psa = emit_psa(0, h)
                    emit_relu(0, h, psa)
                    emit_xd(0, h, psa)

                # All psB products split into 392-wide chunks: each chunk is
                # exactly one PSUM bank, so pool slots recycle at chunk
                # granularity and the psp(p+1) <- exp(p) coupling vanishes.
                CH = [(0, 392), (392, 392)]
                for pr in range(NPAIR):
                    nxt = pr + 1 if pr + 1 < NPAIR else None
                    prompt = sbA.tile([128, L], BF16, tag="prompt")
                    prompts[pr] = prompt
                    pspc = []
                    for n0, nw in CH:
                        psp = psB.tile([128, 392], FP32, tag="psb")
                        pspc.append(psp)
                        nc.tensor.matmul(psp[:], lhsT=mw2d,
                                         rhs=hpairs[pr][:, n0:n0 + nw],
                                         start=True, stop=True)
                        nc.scalar.activation(prompt[:, n0:n0 + nw], psp[:],
                                             AF.Identity, bias=mb2p)

                    psa_n = [None, None]
                    if nxt is not None:
                        emit_xt_dma(nxt)
                        psa_n[0] = emit_psa(nxt, 0)

                    expt = sbA.tile([32, L], BF16, tag="expt")
                    zsum = [None, None]
                    for c, (n0, nw) in enumerate(CH):
                        psm = psB.tile([32, 392], FP32, tag="psb")
                        nc.tensor.matmul(psm[:], lhsT=mtT2,
                                         rhs=prompt[:, n0:n0 + nw],
                                         start=True, stop=True)
                        zsum[c] = smA.tile([32, 1], FP32, tag=f"z{c}",
                                           name=f"zs{pr}_{c}")
                        nc.scalar.activation(expt[:, n0:n0 + nw], psm[:],
                                             AF.Exp, accum_out=zsum[c][:])
                    invz = smA.tile([32, 1], FP32, tag="iz")
                    nc.vector.tensor_tensor(zsum[0][:], zsum[0][:],
                                            zsum[1][:], ALU.add)
                    nc.vector.reciprocal(invz[:], zsum[0][:])

                    if nxt is not None:
                        psa_n[1] = emit_psa(nxt, 1)

                    expn = sbA.tile([32, L], BF16, tag="expn")
                    ftmp = sbA.tile([128, L], BF16, tag="ftmp")
                    for c, (n0, nw) in enumerate(CH):
                        nc.vector.tensor_scalar_mul(expn[:, n0:n0 + nw],
                                                    expt[:, n0:n0 + nw],
                                                    invz[:])
                        pss = psB.tile([128, 392], FP32, tag="psb")
                        nc.tensor.matmul(pss[:], lhsT=ones32,
                                         rhs=expn[:, n0:n0 + nw],
                                         start=True, stop=True)
                        nc.vector.tensor_mul(ftmp[:, n0:n0 + nw],
                                             pss[:],
                                             prompts[pr][:, n0:n0 + nw])
                        nc.vector.reduce_sum(featPQ[c][:, pr:pr + 1],
                                             ftmp[:, n0:n0 + nw],
                                             axis=mybir.AxisListType.X)

                    if nxt is not None:
                        hpairs[nxt] = sbA.tile([128, L], BF16,
                                               name=f"hp{nxt}", tag="h")
                        for h in range(2):
                            emit_relu(nxt, h, psa_n[h])
                            emit_xd(nxt, h, psa_n[h])

                    nc.vector.tensor_tensor(featP[:, pr:pr + 1],
                                            featP[:, pr:pr + 1],
                                            featQ[:, pr:pr + 1], ALU.add)
                    nc.vector.tensor_copy(feat_bf[0:64, 2 * pr:2 * pr + 1],
                                          featP[0:64, pr:pr + 1])
                    nc.vector.tensor_copy(feat_bf[0:64, 2 * pr + 1:2 * pr + 2],
                                          featP[64:128, pr:pr + 1])
                    if pr == NPAIR - 1:
                        # zero "gate": carries a dep on the last phase-A op so
                        # the scheduler can't hoist phase-B gelu (and its act
                        # table load) into phase A
                        nc.vector.tensor_scalar(
                            zgate[:], ftmp[:, 0:1], 0.0, None, ALU.mult)

            # ================= phase H: hypernet conv weights ===============
            # full-partition tiles: the strided-partition scatter reads then
            # stay inside one allocation for the access tracker
            cwalls = [cwp.tile([128, 9 * 512], BF16,
                               name=f"cwall{g}", tag=f"cwall{g}")
                      for g, (_, nb) in enumerate(GROUPS)]
            # pad-gelu for every pair only needs xd + the phase-A gate: emit
            # here so ACT runs them (and the gelu table load) during phase H
            for pr in range(NPAIR):
                pad3s = pad_tiles[pr].rearrange("p (r c) -> p r c", r=30)
                nc.scalar.activation(
                    pad3s[:, 1:29, 1:29],
                    xd_tiles[pr].rearrange("p (a b) -> p a b", a=28)[:],
                    AF.Gelu_apprx_sigmoid, bias=zgate[:])
            with tc.tile_pool(name="psH", bufs=6, space="PSUM") as psH:
                rot = 0
                for n9 in range(9):
                    hwc = hwp.tile([65, 2, 2048], BF16, tag="hw")
                    # Pool SWDGE queue: keeps these 9 transfers off the SP
                    # sequencer (~1.2us each there) and off the shared HWDGE
                    nc.sync.dma_start(out=hwc[:], in_=hw_d[:, :, n9, :])
                    for g, (j8_0, nb) in enumerate(GROUPS):
                        psh = psH.tile([32 * nb, 512], FP32, tag="psh")
                        for a in range(nb):
                            h2, j4 = divmod(j8_0 + a, 4)
                            nc.tensor.matmul(
                                psh[32 * a:32 * a + 32, :],
                                lhsT=feat_bf[:, :],
                                rhs=hwc[:, h2, j4 * 512:(j4 + 1) * 512],
                                start=True, stop=True)
                        dst = cwalls[g][0:32 * nb, n9 * 512:(n9 + 1) * 512]
                        if rot == 0:
                            nc.vector.tensor_copy(dst, psh[:])
                        else:
                            nc.scalar.activation(dst, psh[:], AF.Copy)
                        rot = (rot + 1) % 2

            # pair-major, group-ascending inside a pair: the six scatters the
            # first conv needs come first, ordered to match drain completion.
            # Pairs 0-1 on the SP queue, pairs 2-3 on Pool SWDGE, so neither
            # queue's ~1.1-1.2us/DMA dispatch delays the first convs or the
            # output DMAs queued behind them.
            for pr in range(NPAIR):
                eng = nc.sync
                for g, (j8_0, nb) in enumerate(GROUPS):
                    for h in range(2):
                        s = 2 * pr + h
                        eng.dma_start(
                            out=cw_tiles[pr][64 * h + 8 * j8_0:
                                             64 * h + 8 * (j8_0 + nb), :, h, :],
                            in_=cwalls[g][s:32 * nb:32].rearrange(
                                "p (i k o) -> p i k o", i=8, k=9))

            # ================= phase B: adapter conv + up ===================
            with tc.tile_pool(name="yap", bufs=2) as yap, \
                 tc.tile_pool(name="outp", bufs=3) as outp, \
                 tc.tile_pool(name="psC0", bufs=2, space="PSUM") as psC0, \
                 tc.tile_pool(name="psC1", bufs=2, space="PSUM") as psC1, \
                 tc.tile_pool(name="psU", bufs=2, space="PSUM") as psU:
                rot = 0
                convp = [None] * NPAIR

                def emit_conv(pr, taps):
                    pad3 = pad_tiles[pr].rearrange("p (r c) -> p r c", r=30)
                    if convp[pr] is None:
                        convp[pr] = (psC0.tile([128, 448], FP32, name=f"c0_{pr}", tag="c0"),
                                     psC1.tile([128, 336], FP32, name=f"c1_{pr}", tag="c1"))
                    ps0, ps1 = convp[pr]
                    for k9 in taps:
                        ky, kx = divmod(k9, 3)
                        lw = cw_tiles[pr][:, k9, :, :]
                        nc.tensor.matmul(
                            ps0[:], lhsT=lw,
                            rhs=pad3[:, ky:ky + 16, kx:kx + 28],
                            start=(k9 == 0), stop=(k9 == 8))
                        nc.tensor.matmul(
                            ps1[:], lhsT=lw,
                            rhs=pad3[:, ky + 16:ky + 28, kx:kx + 28],
                            start=(k9 == 0), stop=(k9 == 8))

                def emit_up(pr):
                    nonlocal rot
                    ps0, ps1 = convp[pr]
                    ya = yap.tile([128, L], BF16, tag="ya")
                    nc.scalar.activation(ya[:, 0:448], ps0[:],
                                         AF.Gelu_apprx_sigmoid)
                    nc.scalar.activation(ya[:, 448:784], ps1[:],
                                         AF.Gelu_apprx_sigmoid)
                    last = pr == NPAIR - 1
                    for h in range(2):
                        outt = outp.tile([128, KC, L], BF16, tag="outt")
                        for j3 in range(KC):
                            # one 2-bank psum tile per (h, j3); the two
                            # matmuls split at the bank boundary, one drain
                            psu = psU.tile([128, L], FP32, tag="psu")
                            for n0, nw in NPOS:
                                nc.tensor.matmul(
                                    psu[:, n0:n0 + nw],
                                    lhsT=upw[64 * h:64 * h + 64,
                                             128 * j3:128 * (j3 + 1)],
                                    rhs=ya[64 * h:64 * h + 64, n0:n0 + nw],
                                    start=True, stop=True)
                            dst = outt[:, j3, :]
                            if rot % 3 != 2:    # 2 of 3 on DVE
                                nc.vector.tensor_scalar(
                                    dst, psu[:], upb3[:, j3:j3 + 1],
                                    None, ALU.add)
                            else:
                                nc.scalar.activation(
                                    dst, psu[:], AF.Identity,
                                    bias=upb3[:, j3:j3 + 1])
                            rot += 1
                            if last and h == 1:
                                # chunked final DMA shortens the drain tail
                                nc.sync.dma_start(
                                    out=out_d[2 * pr + h, j3].rearrange(
                                        "p q -> p q"),
                                    in_=outt[:, j3, :])
                        if not (last and h == 1):
                            nc.sync.dma_start(
                                out=out_d[2 * pr + h].rearrange(
                                    "k p q -> p k q"),
                                in_=outt[:])

                # software pipeline: up(p)'s qgelu latency is covered by the
                # first taps of conv(p+1); its drain tail by the rest
                emit_conv(0, range(9))
                for pr in range(1, NPAIR):
                    emit_conv(pr, range(0, 3))
                    emit_up(pr - 1)
                    emit_conv(pr, range(3, 9))
                emit_up(NPAIR - 1)
    nc.finalize()
    return nc


def _prep(x, meta_w1, meta_b1, meta_w2, meta_b2, mask_token,
          hyper_w, hyper_b, down_w, down_b, up_w, up_b):
    f = lambda a: np.ascontiguousarray(np.asarray(a, dtype=np.float32))
    bf = lambda a: np.ascontiguousarray(np.asarray(a).astype(BF16_NP))
    x = f(x)
    xt = x.reshape(B, L, C).transpose(0, 2, 1)            # [B, C, L]
    xt = bf(xt).reshape(B, KC, 128, L).reshape(NCORES, NPAIR, 2, KC, 128, L)

    # psa weights: even parity [meta_w1 | down_w], odd parity swapped
    wA0 = np.concatenate([f(meta_w1), f(down_w)], axis=1)   # [384, 128]
    wA1 = np.concatenate([f(down_w), f(meta_w1)], axis=1)
    wab = np.stack([wA0, wA1]).reshape(2, KC, 128, 128).transpose(
        0, 2, 1, 3).reshape(2, 128, 384)                    # [par, p, (k m)]

    mw2d = np.zeros((128, 128), np.float32)
    mw2d[0:64, 0:64] = f(meta_w2)
    mw2d[64:128, 64:128] = f(meta_w2)
    mtT2s = np.zeros((32, 128), np.float32)     # [m-pair, n-pair]
    mtT2s[0:16, 0:64] = f(mask_token)
    mtT2s[16:32, 64:128] = f(mask_token)
    ones32 = np.zeros((32, 128), np.float32)
    ones32[0:16, 0:64] = 1.0
    ones32[16:32, 64:128] = 1.0
    upw = f(up_w)                                # [64, 384]

    cbw = 2 * 384 + 128 + 32 + 128 + 384
    cb = np.zeros((128, cbw), np.float32)
    cb[:, 0:384] = wab[0]
    cb[:, 384:768] = wab[1]
    cb[:, 768:896] = mw2d
    cb[:, 896:928] = mtT2s.T                    # [128 n-pair, 32 m-pair]
    cb[0:32, 928:1056] = ones32
    cb[0:64, 1056:1440] = upw
    cb[64:128, 1056:1440] = upw                 # dup for base-partition match
    cb = bf(cb)

    fbm = np.zeros((128, 3), np.float32)
    fbm[0:64, 0] = f(meta_b1)
    fbm[64:128, 0] = f(down_b)
    fbm[0:64, 1] = f(down_b)
    fbm[64:128, 1] = f(meta_b1)
    fbm[0:64, 2] = f(meta_b2)
    fbm[64:128, 2] = f(meta_b2)

    upb3 = f(up_b).reshape(KC, 128).T            # [128, 3]

    # hypernet weights: columns packed (j8, i8, ky, kx, o); ones-row = hyper_b
    hw5 = f(hyper_w).reshape(META, DIM, DIM, 3, 3)       # [n, o, i, ky, kx]
    hwc = hw5.transpose(0, 2, 3, 4, 1).reshape(META, 8, 4608)  # [n, j8, (i8 k o)]
    hwc = hwc.reshape(META, 2, 4, 9, 512).transpose(0, 1, 3, 2, 4).reshape(
        META, 2, 9, 2048)
    hb5 = f(hyper_b).reshape(DIM, DIM, 3, 3)             # [o, i, ky, kx]
    hbc = hb5.transpose(1, 2, 3, 0).reshape(8, 4608)     # [j8, (i8 k o)]
    hbc = hbc.reshape(2, 4, 9, 512).transpose(0, 2, 1, 3).reshape(2, 9, 2048)
    hwe = np.concatenate([hwc, hbc[None]], axis=0)       # [65, 2, 9, 2048]
    hwe = bf(hwe)

    consts = {"cb": cb, "fb": fbm, "hw": hwe, "ub": np.ascontiguousarray(upb3)}
    in_maps = []
    for c in range(NCORES):
        m = dict(consts)
        m["xt"] = np.ascontiguousarray(xt[c])
        in_maps.append(m)
    return in_maps


def _run(in_maps, **kw):
    if "nc" not in _CACHE:
        _CACHE["nc"] = _build_nc()
    return run_bass_kernel_spmd(_CACHE["nc"], in_maps, list(range(NCORES)), **kw)


def kernel(x, meta_w1, meta_b1, meta_w2, meta_b2, mask_token,
           hyper_w, hyper_b, down_w, down_b, up_w, up_b, H, W):
    assert int(H) == HH and int(W) == WW
    in_maps = _prep(x, meta_w1, meta_b1, meta_w2, meta_b2, mask_token,
                    hyper_w, hyper_b, down_w, down_b, up_w, up_b)
    res = _run(in_maps)
    outs = []
    for c in range(NCORES):
        o = np.asarray(res.results[c]["out"]).astype(np.float32)
        # [S, KC, 128, L] -> [S, C, L] -> [S, L, C]
        o = o.reshape(S, C, L).transpose(0, 2, 1)
        outs.append(o)
    out = np.concatenate(outs, axis=0)
    return np.ascontiguousarray(out.reshape(B, L, C)).astype(np.float32)


# revision 46
# speedup vs baseline: 4.3632x; 1.0240x over previous
"""Trainium2 Bass kernel for Convpass-swintransformer hypernet-mask adapter.

Data-parallel over batch: 8 NeuronCores x 8 samples each, all weights
replicated. All matmuls run in bf16 (fp32 PSUM accumulation); samples are
processed in pairs with block-diagonal weights so the 64-channel ops fill the
128-wide PE array. Biases are folded into activation/vector drains (or, for
the hypernet, a ones-row on the feat tile) instead of PE bias matmuls.
QuickGELU is a single Gelu_apprx_sigmoid activation; phase-B gelu ops are
gated behind phase A so the ACT function-table isn't thrashed against Exp.
"""
import sys

sys.path.insert(0, "/opt/trn_rl_repo")

import numpy as np

import concourse.bass as bass
import concourse.tile as tile
from concourse import bacc, mybir
from concourse.bass_utils import run_bass_kernel_spmd

AF = mybir.ActivationFunctionType
ALU = mybir.AluOpType
FP32 = mybir.dt.float32
BF16 = mybir.dt.bfloat16
BF16_NP = mybir.dt.np(BF16)

B, L, C = 64, 784, 384
DIM, NM, META = 64, 16, 64
HH, WW = 28, 28
NCORES = 8
S = B // NCORES          # samples per core
NPAIR = S // 2
KC = C // 128            # 3 contraction chunks for C=384
NPOS = [(0, 512), (512, 272)]   # 784 split at psum-bank boundary
NPOSB = [(0, 448), (448, 336)]  # 784 split matching the conv-psum halves
GROUPS = [(0, 3), (3, 3), (6, 2)]   # (first j8, n slots) per hypernet group

_CACHE = {}


def _build_nc():
    nc = bacc.Bacc(None)
    d = nc.declare_dram_parameter
    xt_d = d("xt", [NPAIR, 2, KC, 128, L], BF16, isOutput=False)
    cb_d = d("cb", [128, 2 * 384 + 128 + 32 + 128 + 384], BF16, isOutput=False)
    fb_d = d("fb", [128, 3], FP32, isOutput=False)
    ub_d = d("ub", [128, 3], FP32, isOutput=False)
    hw_d = d("hw", [65, 2, 9, 2048], BF16, isOutput=False)
    out_d = d("out", [S, KC, 128, L], BF16, isOutput=True)

    with tile.TileContext(nc) as tc:
        with tc.tile_pool(name="consts", bufs=1) as cp, \
             tc.tile_pool(name="xdp", bufs=NPAIR) as xdp, \
             tc.tile_pool(name="padp", bufs=NPAIR) as padp, \
             tc.tile_pool(name="cwp", bufs=NPAIR) as cwpp, \
             tc.tile_pool(name="cwall", bufs=1) as cwp, \
             tc.tile_pool(name="hwp", bufs=9) as hwp:
            # ---- constants (two blobs) ----
            cb = cp.tile([128, 2 * 384 + 128 + 32 + 128 + 384], BF16)
            nc.sync.dma_start(out=cb[:], in_=cb_d[:])
            fb = cp.tile([128, 3], FP32)
            upb3 = cp.tile([128, 3], FP32)
            wab = [cb[:, 0:384].rearrange("p (k m) -> p k m", k=KC),
                   cb[:, 384:768].rearrange("p (k m) -> p k m", k=KC)]
            mw2d = cb[:, 768:896]
            mtT2 = cb[:, 896:928]            # [128 n-pair, 32 m-pair]
            ones32 = cb[0:32, 928:1056]
            upw = cb[:, 1056:1440]           # upw dup'd on both halves
            b_a = [fb[:, 0:1], fb[:, 1:2]]   # per-sample-parity psa bias
            mb2p = fb[:, 2:3]

            feat_bf = cp.tile([65, 32], BF16)
            nc.vector.memset(feat_bf[:], 0.0)
            nc.vector.memset(feat_bf[64:65, :], 1.0)
            featP = cp.tile([128, NPAIR], FP32)
            featQ = cp.tile([128, NPAIR], FP32)
            featPQ = [featP, featQ]
            zgate = cp.tile([128, 1], FP32)

            xd_tiles, pad_tiles, cw_tiles = [], [], []
            for pr in range(NPAIR):
                xd = xdp.tile([128, L], BF16)
                xd_tiles.append(xd)
                pad = padp.tile([128, 900], BF16)
                nc.gpsimd.memset(pad[:], 0.0)
                pad_tiles.append(pad)
                cw = cwpp.tile([128, 9, 2, 64], BF16)
                nc.gpsimd.memset(cw[:], 0.0)
                cw_tiles.append(cw)

            # ================= phase A: meta-net / masks / feat =============
            # Software-pipelined: pair p+1's psa matmuls are interleaved
            # between pair p's small matmuls so the PE never drains, and the
            # ACT/DVE queues are emitted critical-path-first.
            with tc.tile_pool(name="xtp", bufs=4) as xtp, \
                 tc.tile_pool(name="psA", bufs=2, space="PSUM") as psA, \
                 tc.tile_pool(name="psB", bufs=4, space="PSUM") as psB, \
                 tc.tile_pool(name="sbA", bufs=2) as sbA, \
                 tc.tile_pool(name="smallA", bufs=2) as smA:
                xts = [None] * NPAIR
                hpairs = [None] * NPAIR
                prompts = [None] * NPAIR

                def emit_xt_dma(pr):
                    xt = xtp.tile([128, 2, KC, L], BF16, name=f"xt{pr}", tag="xt")
                    xts[pr] = xt
                    if pr == 0:
                        for k in range(KC):     # finest grain: start compute
                            nc.sync.dma_start(   # after 1/6 of the data
                                out=xt[:, 0, k, :],
                                in_=xt_d[0, 0, k].rearrange("p q -> p q"))
                        nc.sync.dma_start(
                            out=xt[:, 1, :, :],
                            in_=xt_d[0, 1].rearrange("k p q -> p k q"))
                        nc.sync.dma_start(out=fb[:], in_=fb_d[:])
                        nc.sync.dma_start(out=upb3[:], in_=ub_d[:])
                    else:
                        nc.sync.dma_start(
                            out=xt[:], in_=xt_d[pr].rearrange(
                                "s k p q -> p s k q"))

                def emit_psa(pr, h):
                    psa = psA.tile([128, L], FP32, tag="psa")
                    for n0, nw in NPOS:
                        for k in range(KC):
                            nc.tensor.matmul(
                                psa[:, n0:n0 + nw], lhsT=wab[h][:, k, :],
                                rhs=xts[pr][:, h, k, n0:n0 + nw],
                                start=(k == 0), stop=(k == KC - 1))
                    return psa

                def emit_relu(pr, h, psa):
                    # even parity: h rows 0:64, xd rows 64:128 of psa; odd
                    # parity swapped (host swaps weight columns) so the ACT
                    # relu never shifts partitions
                    nc.scalar.activation(
                        hpairs[pr][64 * h:64 * h + 64, :],
                        psa[64 * h:64 * h + 64, :],
                        AF.Relu, bias=b_a[h][64 * h:64 * h + 64])

                def emit_xd(pr, h, psa):
                    nc.vector.tensor_scalar(
                        xd_tiles[pr][64 * h:64 * h + 64, :],
                        psa[64 - 64 * h:128 - 64 * h, :],
                        b_a[h][64 - 64 * h:128 - 64 * h], None, ALU.add)

                # prologue: pair 0 psa + drains
                emit_xt_dma(0)
                hpairs[0] = sbA.tile([128, L], BF16, name="hp0", tag="h")
                for h in range(2):
                    psa = emit_psa(0, h)
                    emit_relu(0, h, psa)
                    emit_xd(0, h, psa)

                # All psB products split into 392-wide chunks: each chunk is
                # exactly one PSUM bank, so pool slots recycle at chunk
                # granularity and the psp(p+1) <- exp(p) coupling vanishes.
                CH = [(0, 392), (392, 392)]
                for pr in range(NPAIR):
                    nxt = pr + 1 if pr + 1 < NPAIR else None
                    prompt = sbA.tile([128, L], BF16, tag="prompt")
                    prompts[pr] = prompt
                    pspc = []
                    for n0, nw in CH:
                        psp = psB.tile([128, 392], FP32, tag="psb")
                        pspc.append(psp)
                        nc.tensor.matmul(psp[:], lhsT=mw2d,
                                         rhs=hpairs[pr][:, n0:n0 + nw],
                                         start=True, stop=True)
                        nc.scalar.activation(prompt[:, n0:n0 + nw], psp[:],
                                             AF.Identity, bias=mb2p)

                    psa_n = [None, None]
                    if nxt is not None:
                        emit_xt_dma(nxt)
                        psa_n[0] = emit_psa(nxt, 0)

                    expt = sbA.tile([32, L], BF16, tag="expt")
                    zsum = [None, None]
                    for c, (n0, nw) in enumerate(CH):
                        psm = psB.tile([32, 392], FP32, tag="psb")
                        nc.tensor.matmul(psm[:], lhsT=mtT2,
                                         rhs=prompt[:, n0:n0 + nw],
                                         start=True, stop=True)
                        zsum[c] = smA.tile([32, 1], FP32, tag=f"z{c}",
                                           name=f"zs{pr}_{c}")
                        nc.scalar.activation(expt[:, n0:n0 + nw], psm[:],
                                             AF.Exp, accum_out=zsum[c][:])
                    invz = smA.tile([32, 1], FP32, tag="iz")
                    nc.vector.tensor_tensor(zsum[0][:], zsum[0][:],
                                            zsum[1][:], ALU.add)
                    nc.vector.reciprocal(invz[:], zsum[0][:])

                    if nxt is not None:
                        psa_n[1] = emit_psa(nxt, 1)

                    expn = sbA.tile([32, L], BF16, tag="expn")
                    ftmp = sbA.tile([128, L], BF16, tag="ftmp")
                    for c, (n0, nw) in enumerate(CH):
                        nc.vector.tensor_scalar_mul(expn[:, n0:n0 + nw],
                                                    expt[:, n0:n0 + nw],
                                                    invz[:])
                        pss = psB.tile([128, 392], FP32, tag="psb")
                        nc.tensor.matmul(pss[:], lhsT=ones32,
                                         rhs=expn[:, n0:n0 + nw],
                                         start=True, stop=True)
                        nc.vector.tensor_mul(ftmp[:, n0:n0 + nw],
                                             pss[:],
                                             prompts[pr][:, n0:n0 + nw])
                        nc.vector.reduce_sum(featPQ[c][:, pr:pr + 1],
                                             ftmp[:, n0:n0 + nw],
                                             axis=mybir.AxisListType.X)

                    if nxt is not None:
                        hpairs[nxt] = sbA.tile([128, L], BF16,
                                               name=f"hp{nxt}", tag="h")
                        for h in range(2):
                            emit_relu(nxt, h, psa_n[h])
                            emit_xd(nxt, h, psa_n[h])

                    nc.vector.tensor_tensor(featP[:, pr:pr + 1],
                                            featP[:, pr:pr + 1],
                                            featQ[:, pr:pr + 1], ALU.add)
                    nc.vector.tensor_copy(feat_bf[0:64, 2 * pr:2 * pr + 1],
                                          featP[0:64, pr:pr + 1])
                    nc.vector.tensor_copy(feat_bf[0:64, 2 * pr + 1:2 * pr + 2],
                                          featP[64:128, pr:pr + 1])
                    if pr == NPAIR - 1:
                        # zero "gate": carries a dep on the last phase-A op so
                        # the scheduler can't hoist phase-B gelu (and its act
                        # table load) into phase A
                        nc.vector.tensor_scalar(
                            zgate[:], ftmp[:, 0:1], 0.0, None, ALU.mult)

            # ================= phase H: hypernet conv weights ===============
            # full-partition tiles: the strided-partition scatter reads then
            # stay inside one allocation for the access tracker
            cwalls = [cwp.tile([128, 9 * 512], BF16,
                               name=f"cwall{g}", tag=f"cwall{g}")
                      for g, (_, nb) in enumerate(GROUPS)]
            # pad-gelu for every pair only needs xd + the phase-A gate: emit
            # here so ACT runs them (and the gelu table load) during phase H
            for pr in range(NPAIR):
                pad3s = pad_tiles[pr].rearrange("p (r c) -> p r c", r=30)
                nc.scalar.activation(
                    pad3s[:, 1:29, 1:29],
                    xd_tiles[pr].rearrange("p (a b) -> p a b", a=28)[:],
                    AF.Gelu_apprx_sigmoid, bias=zgate[:])
            with tc.tile_pool(name="psH", bufs=6, space="PSUM") as psH:
                rot = 0
                for n9 in range(9):
                    hwc = hwp.tile([65, 2, 2048], BF16, tag="hw")
                    # Pool SWDGE queue: keeps these 9 transfers off the SP
                    # sequencer (~1.2us each there) and off the shared HWDGE
                    nc.gpsimd.dma_start(out=hwc[:], in_=hw_d[:, :, n9, :])
                    for g, (j8_0, nb) in enumerate(GROUPS):
                        psh = psH.tile([32 * nb, 512], FP32, tag="psh")
                        for a in range(nb):
                            h2, j4 = divmod(j8_0 + a, 4)
                            nc.tensor.matmul(
                                psh[32 * a:32 * a + 32, :],
                                lhsT=feat_bf[:, :],
                                rhs=hwc[:, h2, j4 * 512:(j4 + 1) * 512],
                                start=True, stop=True)
                        dst = cwalls[g][0:32 * nb, n9 * 512:(n9 + 1) * 512]
                        if rot == 0:
                            nc.vector.tensor_copy(dst, psh[:])
                        else:
                            nc.scalar.activation(dst, psh[:], AF.Copy)
                        rot = (rot + 1) % 2

            # pair-major, group-ascending inside a pair: the six scatters the
            # first conv needs come first, ordered to match drain completion.
            # Pairs 0-1 on the SP queue, pairs 2-3 on Pool SWDGE, so neither
            # queue's ~1.1-1.2us/DMA dispatch delays the first convs or the
            # output DMAs queued behind them.
            for pr in range(NPAIR):
                eng = nc.sync if pr < 1 else nc.gpsimd
                for g, (j8_0, nb) in enumerate(GROUPS):
                    for h in range(2):
                        s = 2 * pr + h
                        eng.dma_start(
                            out=cw_tiles[pr][64 * h + 8 * j8_0:
                                             64 * h + 8 * (j8_0 + nb), :, h, :],
                            in_=cwalls[g][s:32 * nb:32].rearrange(
                                "p (i k o) -> p i k o", i=8, k=9))

            # ================= phase B: adapter conv + up ===================
            with tc.tile_pool(name="yap", bufs=2) as yap, \
                 tc.tile_pool(name="outp", bufs=3) as outp, \
                 tc.tile_pool(name="psC0", bufs=2, space="PSUM") as psC0, \
                 tc.tile_pool(name="psC1", bufs=2, space="PSUM") as psC1, \
                 tc.tile_pool(name="psU", bufs=2, space="PSUM") as psU:
                rot = 0
                convp = [None] * NPAIR

                def emit_conv(pr, taps):
                    pad3 = pad_tiles[pr].rearrange("p (r c) -> p r c", r=30)
                    if convp[pr] is None:
                        convp[pr] = (psC0.tile([128, 448], FP32, name=f"c0_{pr}", tag="c0"),
                                     psC1.tile([128, 336], FP32, name=f"c1_{pr}", tag="c1"))
                    ps0, ps1 = convp[pr]
                    for k9 in taps:
                        ky, kx = divmod(k9, 3)
                        lw = cw_tiles[pr][:, k9, :, :]
                        nc.tensor.matmul(
                            ps0[:], lhsT=lw,
                            rhs=pad3[:, ky:ky + 16, kx:kx + 28],
                            start=(k9 == 0), stop=(k9 == 8))
                        nc.tensor.matmul(
                            ps1[:], lhsT=lw,
                            rhs=pad3[:, ky + 16:ky + 28, kx:kx + 28],
                            start=(k9 == 0), stop=(k9 == 8))

                def emit_up(pr):
                    nonlocal rot
                    ps0, ps1 = convp[pr]
                    ya = yap.tile([128, L], BF16, tag="ya")
                    nc.scalar.activation(ya[:, 0:448], ps0[:],
                                         AF.Gelu_apprx_sigmoid)
                    nc.scalar.activation(ya[:, 448:784], ps1[:],
                                         AF.Gelu_apprx_sigmoid)
                    last = pr == NPAIR - 1
                    for h in range(2):
                        outt = outp.tile([128, KC, L], BF16, tag="outt")
                        for j3 in range(KC):
                            # one 2-bank psum tile per (h, j3); the two
                            # matmuls split at the bank boundary, one drain
                            psu = psU.tile([128, L], FP32, tag="psu")
                            for n0, nw in NPOS:
                                nc.tensor.matmul(
                                    psu[:, n0:n0 + nw],
                                    lhsT=upw[64 * h:64 * h + 64,
                                             128 * j3:128 * (j3 + 1)],
                                    rhs=ya[64 * h:64 * h + 64, n0:n0 + nw],
                                    start=True, stop=True)
                            dst = outt[:, j3, :]
                            if rot % 3 != 2:    # 2 of 3 on DVE
                                nc.vector.tensor_scalar(
                                    dst, psu[:], upb3[:, j3:j3 + 1],
                                    None, ALU.add)
                            else:
                                nc.scalar.activation(
                                    dst, psu[:], AF.Identity,
                                    bias=upb3[:, j3:j3 + 1])
                            rot += 1
                            if last and h == 1:
                                # chunked final DMA shortens the drain tail
                                nc.sync.dma_start(
                                    out=out_d[2 * pr + h, j3].rearrange(
                                        "p q -> p q"),
                                    in_=outt[:, j3, :])
                        if not (last and h == 1):
                            nc.sync.dma_start(
                                out=out_d[2 * pr + h].rearrange(
                                    "k p q -> p k q"),
                                in_=outt[:])

                # software pipeline: up(p)'s qgelu latency is covered by the
                # first taps of conv(p+1); its drain tail by the rest
                emit_conv(0, range(9))
                for pr in range(1, NPAIR):
                    emit_conv(pr, range(0, 3))
                    emit_up(pr - 1)
                    emit_conv(pr, range(3, 9))
                emit_up(NPAIR - 1)
    nc.finalize()
    return nc


def _prep(x, meta_w1, meta_b1, meta_w2, meta_b2, mask_token,
          hyper_w, hyper_b, down_w, down_b, up_w, up_b):
    f = lambda a: np.ascontiguousarray(np.asarray(a, dtype=np.float32))
    bf = lambda a: np.ascontiguousarray(np.asarray(a).astype(BF16_NP))
    x = f(x)
    xt = x.reshape(B, L, C).transpose(0, 2, 1)            # [B, C, L]
    xt = bf(xt).reshape(B, KC, 128, L).reshape(NCORES, NPAIR, 2, KC, 128, L)

    # psa weights: even parity [meta_w1 | down_w], odd parity swapped
    wA0 = np.concatenate([f(meta_w1), f(down_w)], axis=1)   # [384, 128]
    wA1 = np.concatenate([f(down_w), f(meta_w1)], axis=1)
    wab = np.stack([wA0, wA1]).reshape(2, KC, 128, 128).transpose(
        0, 2, 1, 3).reshape(2, 128, 384)                    # [par, p, (k m)]

    mw2d = np.zeros((128, 128), np.float32)
    mw2d[0:64, 0:64] = f(meta_w2)
    mw2d[64:128, 64:128] = f(meta_w2)
    mtT2s = np.zeros((32, 128), np.float32)     # [m-pair, n-pair]
    mtT2s[0:16, 0:64] = f(mask_token)
    mtT2s[16:32, 64:128] = f(mask_token)
    ones32 = np.zeros((32, 128), np.float32)
    ones32[0:16, 0:64] = 1.0
    ones32[16:32, 64:128] = 1.0
    upw = f(up_w)                                # [64, 384]

    cbw = 2 * 384 + 128 + 32 + 128 + 384
    cb = np.zeros((128, cbw), np.float32)
    cb[:, 0:384] = wab[0]
    cb[:, 384:768] = wab[1]
    cb[:, 768:896] = mw2d
    cb[:, 896:928] = mtT2s.T                    # [128 n-pair, 32 m-pair]
    cb[0:32, 928:1056] = ones32
    cb[0:64, 1056:1440] = upw
    cb[64:128, 1056:1440] = upw                 # dup for base-partition match
    cb = bf(cb)

    fbm = np.zeros((128, 3), np.float32)
    fbm[0:64, 0] = f(meta_b1)
    fbm[64:128, 0] = f(down_b)
    fbm[0:64, 1] = f(down_b)
    fbm[64:128, 1] = f(meta_b1)
    fbm[0:64, 2] = f(meta_b2)
    fbm[64:128, 2] = f(meta_b2)

    upb3 = f(up_b).reshape(KC, 128).T            # [128, 3]

    # hypernet weights: columns packed (j8, i8, ky, kx, o); ones-row = hyper_b
    hw5 = f(hyper_w).reshape(META, DIM, DIM, 3, 3)       # [n, o, i, ky, kx]
    hwc = hw5.transpose(0, 2, 3, 4, 1).reshape(META, 8, 4608)  # [n, j8, (i8 k o)]
    hwc = hwc.reshape(META, 2, 4, 9, 512).transpose(0, 1, 3, 2, 4).reshape(
        META, 2, 9, 2048)
    hb5 = f(hyper_b).reshape(DIM, DIM, 3, 3)             # [o, i, ky, kx]
    hbc = hb5.transpose(1, 2, 3, 0).reshape(8, 4608)     # [j8, (i8 k o)]
    hbc = hbc.reshape(2, 4, 9, 512).transpose(0, 2, 1, 3).reshape(2, 9, 2048)
    hwe = np.concatenate([hwc, hbc[None]], axis=0)       # [65, 2, 9, 2048]
    hwe = bf(hwe)

    consts = {"cb": cb, "fb": fbm, "hw": hwe, "ub": np.ascontiguousarray(upb3)}
    in_maps = []
    for c in range(NCORES):
        m = dict(consts)
        m["xt"] = np.ascontiguousarray(xt[c])
        in_maps.append(m)
    return in_maps


def _run(in_maps, **kw):
    if "nc" not in _CACHE:
        _CACHE["nc"] = _build_nc()
    return run_bass_kernel_spmd(_CACHE["nc"], in_maps, list(range(NCORES)), **kw)


def kernel(x, meta_w1, meta_b1, meta_w2, meta_b2, mask_token,
           hyper_w, hyper_b, down_w, down_b, up_w, up_b, H, W):
    assert int(H) == HH and int(W) == WW
    in_maps = _prep(x, meta_w1, meta_b1, meta_w2, meta_b2, mask_token,
                    hyper_w, hyper_b, down_w, down_b, up_w, up_b)
    res = _run(in_maps)
    outs = []
    for c in range(NCORES):
        o = np.asarray(res.results[c]["out"]).astype(np.float32)
        # [S, KC, 128, L] -> [S, C, L] -> [S, L, C]
        o = o.reshape(S, C, L).transpose(0, 2, 1)
        outs.append(o)
    out = np.concatenate(outs, axis=0)
    return np.ascontiguousarray(out.reshape(B, L, C)).astype(np.float32)


# revision 57
# speedup vs baseline: 4.3763x; 1.0030x over previous
"""Trainium2 Bass kernel for Convpass-swintransformer hypernet-mask adapter.

Data-parallel over batch: 8 NeuronCores x 8 samples each, all weights
replicated. All matmuls run in bf16 (fp32 PSUM accumulation); samples are
processed in pairs with block-diagonal weights so the 64-channel ops fill the
128-wide PE array. Biases are folded into activation/vector drains (or, for
the hypernet, a ones-row on the feat tile) instead of PE bias matmuls.
QuickGELU is a single Gelu_apprx_sigmoid activation; phase-B gelu ops are
gated behind phase A so the ACT function-table isn't thrashed against Exp.
"""
import sys

sys.path.insert(0, "/opt/trn_rl_repo")

import numpy as np

import concourse.bass as bass
import concourse.tile as tile
from concourse import bacc, mybir
from concourse.bass_utils import run_bass_kernel_spmd

AF = mybir.ActivationFunctionType
ALU = mybir.AluOpType
FP32 = mybir.dt.float32
BF16 = mybir.dt.bfloat16
BF16_NP = mybir.dt.np(BF16)

B, L, C = 64, 784, 384
DIM, NM, META = 64, 16, 64
HH, WW = 28, 28
NCORES = 8
S = B // NCORES          # samples per core
NPAIR = S // 2
KC = C // 128            # 3 contraction chunks for C=384
NPOS = [(0, 512), (512, 272)]   # 784 split at psum-bank boundary
NPOSB = [(0, 448), (448, 336)]  # 784 split matching the conv-psum halves
GROUPS = [(0, 3), (3, 3), (6, 2)]   # (first j8, n slots) per hypernet group

_CACHE = {}


def _build_nc():
    nc = bacc.Bacc(None)
    d = nc.declare_dram_parameter
    xt_d = d("xt", [NPAIR, 2, KC, 128, L], BF16, isOutput=False)
    cb_d = d("cb", [128, 2 * 384 + 64 + 32 + 128 + 384], BF16, isOutput=False)
    fb_d = d("fb", [128, 5], FP32, isOutput=False)
    ub_d = d("ub", [128, 3], FP32, isOutput=False)
    hw_d = d("hw", [65, 2, 9, 2048], BF16, isOutput=False)
    out_d = d("out", [S, KC, 128, L], BF16, isOutput=True)

    with tile.TileContext(nc) as tc:
        with tc.tile_pool(name="consts", bufs=1) as cp, \
             tc.tile_pool(name="hxp", bufs=2 * NPAIR) as hxp, \
             tc.tile_pool(name="padp", bufs=NPAIR) as padp, \
             tc.tile_pool(name="cwp", bufs=NPAIR) as cwpp, \
             tc.tile_pool(name="cwall", bufs=1) as cwp, \
             tc.tile_pool(name="hwp", bufs=9) as hwp:
            # ---- constants (two blobs) ----
            cb = cp.tile([128, 2 * 384 + 64 + 32 + 128 + 384], BF16)
            fb = cp.tile([128, 5], FP32)
            upb3 = cp.tile([128, 3], FP32)
            wab = [cb[:, 0:384].rearrange("p (k m) -> p k m", k=KC),
                   cb[:, 384:768].rearrange("p (k m) -> p k m", k=KC)]
            mw2dup = cb[:, 768:832]          # meta_w2 dup'd on both halves
            mtT2 = cb[:, 832:864]            # [128 n-pair, 32 m-pair]
            ones32 = cb[0:32, 864:992]
            upw = cb[:, 992:1376]            # upw dup'd on both halves
            b_a = [fb[:, 0:1], fb[:, 1:2]]   # per-sample-parity psa bias
            mb2p = fb[:, 2:3]
            bnd = [fb[:, 3:4], fb[:, 4:5]]   # relu bound: 0 on h-rows,
                                             # -3e38 on xd-rows, per parity

            feat_bf = cp.tile([65, 32], BF16)
            nc.vector.memset(feat_bf[:], 0.0)
            nc.vector.memset(feat_bf[64:65, :], 1.0)
            featP = cp.tile([128, NPAIR], FP32)
            featQ = cp.tile([128, NPAIR], FP32)
            featPQ = [featP, featQ]
            zgate = cp.tile([128, 1], FP32)

            hx_tiles, pad_tiles, cw_tiles = [], [], []
            for s in range(S):
                hx = hxp.tile([128, L], BF16, name=f"hx{s}", tag="hx")
                hx_tiles.append(hx)
            for pr in range(NPAIR):
                pad = padp.tile([128, 900], BF16)
                nc.gpsimd.memset(pad[:], 0.0)
                pad_tiles.append(pad)
                cw = cwpp.tile([128, 9, 2, 64], BF16)
                nc.gpsimd.memset(cw[:], 0.0)
                cw_tiles.append(cw)

            # ================= phase A: meta-net / masks / feat =============
            # Software-pipelined: pair p+1's psa matmuls are interleaved
            # between pair p's small matmuls so the PE never drains, and the
            # ACT/DVE queues are emitted critical-path-first.
            with tc.tile_pool(name="xtp", bufs=4) as xtp, \
                 tc.tile_pool(name="psA", bufs=2, space="PSUM") as psA, \
                 tc.tile_pool(name="psB", bufs=4, space="PSUM") as psB, \
                 tc.tile_pool(name="sbA", bufs=2) as sbA, \
                 tc.tile_pool(name="smallA", bufs=2) as smA:
                xts = [None] * NPAIR
                prompts = [None] * NPAIR

                def emit_xt_dma(pr):
                    xt = xtp.tile([128, 2, KC, L], BF16, name=f"xt{pr}", tag="xt")
                    xts[pr] = xt
                    if pr == 0:
                        nc.sync.dma_start(   # first compute chunk leads
                            out=xt[:, 0, 0, :],
                            in_=xt_d[0, 0, 0].rearrange("p q -> p q"))
                        nc.sync.dma_start(out=cb[:], in_=cb_d[:])
                        for k in range(1, KC):
                            nc.sync.dma_start(
                                out=xt[:, 0, k, :],
                                in_=xt_d[0, 0, k].rearrange("p q -> p q"))
                        nc.sync.dma_start(
                            out=xt[:, 1, :, :],
                            in_=xt_d[0, 1].rearrange("k p q -> p k q"))
                        nc.sync.dma_start(out=fb[:], in_=fb_d[:])
                        nc.sync.dma_start(out=upb3[:], in_=ub_d[:])
                    else:
                        nc.sync.dma_start(
                            out=xt[:], in_=xt_d[pr].rearrange(
                                "s k p q -> p s k q"))

                def emit_psa(pr, h):
                    psa = psA.tile([128, L], FP32, tag="psa")
                    for n0, nw in NPOS:
                        for k in range(KC):
                            nc.tensor.matmul(
                                psa[:, n0:n0 + nw], lhsT=wab[h][:, k, :],
                                rhs=xts[pr][:, h, k, n0:n0 + nw],
                                start=(k == 0), stop=(k == KC - 1))
                    return psa

                def emit_hx(pr, h, psa):
                    # one fused drain per sample: (psa + bias) max bound,
                    # where bound is 0 on the h-rows (= relu) and -3e38 on
                    # the xd-rows (= identity). h-part at rows 64h, xd-part
                    # at rows 64*(1-h); phase B flips h to match.
                    nc.vector.tensor_scalar(
                        hx_tiles[2 * pr + h][:], psa[:],
                        b_a[h], bnd[h], ALU.add, ALU.max)

                # prologue: pair 0 psa + drains
                emit_xt_dma(0)
                for h in range(2):
                    psa = emit_psa(0, h)
                    emit_hx(0, h, psa)

                # All psB products split into 392-wide chunks: each chunk is
                # exactly one PSUM bank, so pool slots recycle at chunk
                # granularity and the psp(p+1) <- exp(p) coupling vanishes.
                CH = [(0, 392), (392, 392)]
                for pr in range(NPAIR):
                    nxt = pr + 1 if pr + 1 < NPAIR else None
                    prompt = sbA.tile([128, L], BF16, tag="prompt")
                    prompts[pr] = prompt
                    for n0, nw in CH:
                        psp = psB.tile([128, 392], FP32, tag="psb")
                        for h in range(2):
                            q0 = 64 * h     # h-part rows of sample 2pr+h
                            nc.tensor.matmul(
                                psp[q0:q0 + 64, :],
                                lhsT=mw2dup[q0:q0 + 64, :],
                                rhs=hx_tiles[2 * pr + h][q0:q0 + 64,
                                                         n0:n0 + nw],
                                start=True, stop=True)
                        nc.scalar.activation(prompt[:, n0:n0 + nw],
                                             psp[:], AF.Identity,
                                             bias=mb2p)

                    psa_n = [None, None]
                    if nxt is not None:
                        emit_xt_dma(nxt)
                        psa_n[0] = emit_psa(nxt, 0)

                    expt = sbA.tile([32, L], BF16, tag="expt")
                    zsum = [None, None]
                    for c, (n0, nw) in enumerate(CH):
                        psm = psB.tile([32, 392], FP32, tag="psb")
                        nc.tensor.matmul(psm[:], lhsT=mtT2,
                                         rhs=prompt[:, n0:n0 + nw],
                                         start=True, stop=True)
                        zsum[c] = smA.tile([32, 1], FP32, tag=f"z{c}",
                                           name=f"zs{pr}_{c}")
                        nc.scalar.activation(expt[:, n0:n0 + nw], psm[:],
                                             AF.Exp, accum_out=zsum[c][:])
                    invz = smA.tile([32, 1], FP32, tag="iz")
                    nc.vector.tensor_tensor(zsum[0][:], zsum[0][:],
                                            zsum[1][:], ALU.add)
                    nc.vector.reciprocal(invz[:], zsum[0][:])

                    if nxt is not None:
                        psa_n[1] = emit_psa(nxt, 1)

                    expn = sbA.tile([32, L], BF16, tag="expn")
                    ftmp = sbA.tile([128, L], BF16, tag="ftmp")
                    for c, (n0, nw) in enumerate(CH):
                        nc.vector.tensor_scalar_mul(expn[:, n0:n0 + nw],
                                                    expt[:, n0:n0 + nw],
                                                    invz[:])
                        pss = psB.tile([128, 392], FP32, tag="psb")
                        nc.tensor.matmul(pss[:], lhsT=ones32,
                                         rhs=expn[:, n0:n0 + nw],
                                         start=True, stop=True)
                        nc.vector.tensor_mul(ftmp[:, n0:n0 + nw],
                                             pss[:],
                                             prompts[pr][:, n0:n0 + nw])
                        junk = sbA.tile([128, 392], BF16, tag="junk")
                        nc.scalar.activation(junk[:], ftmp[:, n0:n0 + nw],
                                             AF.Identity,
                                             accum_out=featPQ[c][:, pr:pr + 1])

                    if nxt is not None:
                        for h in range(2):
                            emit_hx(nxt, h, psa_n[h])

                    nc.vector.tensor_tensor(featP[:, pr:pr + 1],
                                            featP[:, pr:pr + 1],
                                            featQ[:, pr:pr + 1], ALU.add)
                    nc.vector.tensor_copy(feat_bf[0:64, 2 * pr:2 * pr + 1],
                                          featP[0:64, pr:pr + 1])
                    nc.vector.tensor_copy(feat_bf[0:64, 2 * pr + 1:2 * pr + 2],
                                          featP[64:128, pr:pr + 1])
                    if pr == NPAIR - 1:
                        # zero "gate": carries a dep on the last phase-A op so
                        # the scheduler can't hoist phase-B gelu (and its act
                        # table load) into phase A
                        nc.vector.tensor_scalar(
                            zgate[:], ftmp[:, 0:1], 0.0, None, ALU.mult)

            # ================= phase H: hypernet conv weights ===============
            # full-partition tiles: the strided-partition scatter reads then
            # stay inside one allocation for the access tracker
            cwalls = [cwp.tile([128, 9 * 512], BF16,
                               name=f"cwall{g}", tag=f"cwall{g}")
                      for g, (_, nb) in enumerate(GROUPS)]
            # pad-gelu for every pair only needs xd + the phase-A gate: emit
            # here so ACT runs them (and the gelu table load) during phase H
            for pr in range(NPAIR):
                pad3s = pad_tiles[pr].rearrange("p (r c) -> p r c", r=30)
                for h in range(2):
                    q0 = 64 - 64 * h    # xd rows of sample 2pr+h
                    nc.scalar.activation(
                        pad3s[q0:q0 + 64, 1:29, 1:29],
                        hx_tiles[2 * pr + h][q0:q0 + 64, :].rearrange(
                            "p (a b) -> p a b", a=28)[:],
                        AF.Gelu_apprx_sigmoid, bias=zgate[q0:q0 + 64])
            with tc.tile_pool(name="psH", bufs=6, space="PSUM") as psH:
                rot = 0
                for n9 in range(9):
                    hwc = hwp.tile([65, 2, 2048], BF16, tag="hw")
                    # Pool SWDGE queue: keeps these 9 transfers off the SP
                    # sequencer (~1.2us each there) and off the shared HWDGE
                    nc.gpsimd.dma_start(out=hwc[:], in_=hw_d[:, :, n9, :])
                    for g, (j8_0, nb) in enumerate(GROUPS):
                        psh = psH.tile([32 * nb, 512], FP32, tag="psh")
                        for a in range(nb):
                            h2, j4 = divmod(j8_0 + a, 4)
                            nc.tensor.matmul(
                                psh[32 * a:32 * a + 32, :],
                                lhsT=feat_bf[:, :],
                                rhs=hwc[:, h2, j4 * 512:(j4 + 1) * 512],
                                start=True, stop=True)
                        dst = cwalls[g][0:32 * nb, n9 * 512:(n9 + 1) * 512]
                        if rot == 0:
                            nc.vector.tensor_copy(dst, psh[:])
                        else:
                            nc.scalar.activation(dst, psh[:], AF.Copy)
                        rot = (rot + 1) % 2

            # pair-major, group-ascending inside a pair: the six scatters the
            # first conv needs come first, ordered to match drain completion.
            # Pairs 0-1 on the SP queue, pairs 2-3 on Pool SWDGE, so neither
            # queue's ~1.1-1.2us/DMA dispatch delays the first convs or the
            # output DMAs queued behind them.
            for pr in range(NPAIR):
                for g, (j8_0, nb) in enumerate(GROUPS):
                    for h in range(2):
                        s = 2 * pr + h
                        q = 1 - h       # storage half (matches xd layout)
                        eng = nc.sync if h == 0 else nc.gpsimd
                        eng.dma_start(
                            out=cw_tiles[pr][64 * q + 8 * j8_0:
                                             64 * q + 8 * (j8_0 + nb), :, q, :],
                            in_=cwalls[g][s:32 * nb:32].rearrange(
                                "p (i k o) -> p i k o", i=8, k=9))

            # ================= phase B: adapter conv + up ===================
            with tc.tile_pool(name="yap", bufs=2) as yap, \
                 tc.tile_pool(name="outp", bufs=3) as outp, \
                 tc.tile_pool(name="psC0", bufs=2, space="PSUM") as psC0, \
                 tc.tile_pool(name="psC1", bufs=2, space="PSUM") as psC1, \
                 tc.tile_pool(name="psU", bufs=2, space="PSUM") as psU:
                rot = 0
                convp = [None] * NPAIR

                def emit_conv(pr, taps):
                    pad3 = pad_tiles[pr].rearrange("p (r c) -> p r c", r=30)
                    if convp[pr] is None:
                        convp[pr] = (psC0.tile([128, 448], FP32, name=f"c0_{pr}", tag="c0"),
                                     psC1.tile([128, 336], FP32, name=f"c1_{pr}", tag="c1"))
                    ps0, ps1 = convp[pr]
                    for k9 in taps:
                        ky, kx = divmod(k9, 3)
                        lw = cw_tiles[pr][:, k9, :, :]
                        nc.tensor.matmul(
                            ps0[:], lhsT=lw,
                            rhs=pad3[:, ky:ky + 16, kx:kx + 28],
                            start=(k9 == 0), stop=(k9 == 8))
                        nc.tensor.matmul(
                            ps1[:], lhsT=lw,
                            rhs=pad3[:, ky + 16:ky + 28, kx:kx + 28],
                            start=(k9 == 0), stop=(k9 == 8))

                def emit_up(pr):
                    nonlocal rot
                    ps0, ps1 = convp[pr]
                    ya = yap.tile([128, L], BF16, tag="ya")
                    nc.scalar.activation(ya[:, 0:448], ps0[:],
                                         AF.Gelu_apprx_sigmoid)
                    nc.scalar.activation(ya[:, 448:784], ps1[:],
                                         AF.Gelu_apprx_sigmoid)
                    last = pr == NPAIR - 1
                    for q in range(2):
                        h = 1 - q       # sample parity stored in half q
                        outt = outp.tile([128, KC, L], BF16, tag="outt")
                        for j3 in range(KC):
                            # one 2-bank psum tile per (h, j3); the two
                            # matmuls split at the bank boundary, one drain
                            psu = psU.tile([128, L], FP32, tag="psu")
                            for n0, nw in NPOS:
                                nc.tensor.matmul(
                                    psu[:, n0:n0 + nw],
                                    lhsT=upw[64 * q:64 * q + 64,
                                             128 * j3:128 * (j3 + 1)],
                                    rhs=ya[64 * q:64 * q + 64, n0:n0 + nw],
                                    start=True, stop=True)
                            dst = outt[:, j3, :]
                            if rot % 3 != 2:    # 2 of 3 on DVE
                                nc.vector.tensor_scalar(
                                    dst, psu[:], upb3[:, j3:j3 + 1],
                                    None, ALU.add)
                            else:
                                nc.scalar.activation(
                                    dst, psu[:], AF.Identity,
                                    bias=upb3[:, j3:j3 + 1])
                            rot += 1
                            if last and q == 0:
                                # chunked final DMA shortens the drain tail
                                nc.sync.dma_start(
                                    out=out_d[2 * pr + h, j3].rearrange(
                                        "p q -> p q"),
                                    in_=outt[:, j3, :])
                        if not (last and q == 0):
                            nc.sync.dma_start(
                                out=out_d[2 * pr + h].rearrange(
                                    "k p q -> p k q"),
                                in_=outt[:])

                # software pipeline: up(p)'s qgelu latency is covered by the
                # first taps of conv(p+1); its drain tail by the rest
                emit_conv(0, range(9))
                for pr in range(1, NPAIR):
                    emit_conv(pr, range(0, 3))
                    emit_up(pr - 1)
                    emit_conv(pr, range(3, 9))
                emit_up(NPAIR - 1)
    nc.finalize()
    return nc


def _prep(x, meta_w1, meta_b1, meta_w2, meta_b2, mask_token,
          hyper_w, hyper_b, down_w, down_b, up_w, up_b):
    f = lambda a: np.ascontiguousarray(np.asarray(a, dtype=np.float32))
    bf = lambda a: np.ascontiguousarray(np.asarray(a).astype(BF16_NP))
    x = f(x)
    xt = x.reshape(B, L, C).transpose(0, 2, 1)            # [B, C, L]
    xt = bf(xt).reshape(B, KC, 128, L).reshape(NCORES, NPAIR, 2, KC, 128, L)

    # psa weights: even parity [meta_w1 | down_w], odd parity swapped
    wA0 = np.concatenate([f(meta_w1), f(down_w)], axis=1)   # [384, 128]
    wA1 = np.concatenate([f(down_w), f(meta_w1)], axis=1)
    wab = np.stack([wA0, wA1]).reshape(2, KC, 128, 128).transpose(
        0, 2, 1, 3).reshape(2, 128, 384)                    # [par, p, (k m)]

    mtT2s = np.zeros((32, 128), np.float32)     # [m-pair, n-pair]
    mtT2s[0:16, 0:64] = f(mask_token)
    mtT2s[16:32, 64:128] = f(mask_token)
    ones32 = np.zeros((32, 128), np.float32)
    ones32[0:16, 0:64] = 1.0
    ones32[16:32, 64:128] = 1.0
    upw = f(up_w)                                # [64, 384]

    cbw = 2 * 384 + 64 + 32 + 128 + 384
    cb = np.zeros((128, cbw), np.float32)
    cb[:, 0:384] = wab[0]
    cb[:, 384:768] = wab[1]
    cb[0:64, 768:832] = f(meta_w2)
    cb[64:128, 768:832] = f(meta_w2)            # dup for base-partition match
    cb[:, 832:864] = mtT2s.T                    # [128 n-pair, 32 m-pair]
    cb[0:32, 864:992] = ones32
    cb[0:64, 992:1376] = upw
    cb[64:128, 992:1376] = upw                  # dup for base-partition match
    cb = bf(cb)

    fbm = np.zeros((128, 5), np.float32)
    fbm[0:64, 0] = f(meta_b1)
    fbm[64:128, 0] = f(down_b)
    fbm[0:64, 1] = f(down_b)
    fbm[64:128, 1] = f(meta_b1)
    fbm[0:64, 2] = f(meta_b2)
    fbm[64:128, 2] = f(meta_b2)
    fbm[0:64, 3] = 0.0                          # relu bound, parity 0
    fbm[64:128, 3] = -3.0e38
    fbm[0:64, 4] = -3.0e38                      # relu bound, parity 1
    fbm[64:128, 4] = 0.0

    upb3 = f(up_b).reshape(KC, 128).T            # [128, 3]

    # hypernet weights: columns packed (j8, i8, ky, kx, o); ones-row = hyper_b
    hw5 = f(hyper_w).reshape(META, DIM, DIM, 3, 3)       # [n, o, i, ky, kx]
    hwc = hw5.transpose(0, 2, 3, 4, 1).reshape(META, 8, 4608)  # [n, j8, (i8 k o)]
    hwc = hwc.reshape(META, 2, 4, 9, 512).transpose(0, 1, 3, 2, 4).reshape(
        META, 2, 9, 2048)
    hb5 = f(hyper_b).reshape(DIM, DIM, 3, 3)             # [o, i, ky, kx]
    hbc = hb5.transpose(1, 2, 3, 0).reshape(8, 4608)     # [j8, (i8 k o)]
    hbc = hbc.reshape(2, 4, 9, 512).transpose(0, 2, 1, 3).reshape(2, 9, 2048)
    hwe = np.concatenate([hwc, hbc[None]], axis=0)       # [65, 2, 9, 2048]
    hwe = bf(hwe)

    consts = {"cb": cb, "fb": fbm, "hw": hwe, "ub": np.ascontiguousarray(upb3)}
    in_maps = []
    for c in range(NCORES):
        m = dict(consts)
        m["xt"] = np.ascontiguousarray(xt[c])
        in_maps.append(m)
    return in_maps


def _run(in_maps, **kw):
    if "nc" not in _CACHE:
        _CACHE["nc"] = _build_nc()
    return run_bass_kernel_spmd(_CACHE["nc"], in_maps, list(range(NCORES)), **kw)


def kernel(x, meta_w1, meta_b1, meta_w2, meta_b2, mask_token,
           hyper_w, hyper_b, down_w, down_b, up_w, up_b, H, W):
    assert int(H) == HH and int(W) == WW
    in_maps = _prep(x, meta_w1, meta_b1, meta_w2, meta_b2, mask_token,
                    hyper_w, hyper_b, down_w, down_b, up_w, up_b)
    res = _run(in_maps)
    outs = []
    for c in range(NCORES):
        o = np.asarray(res.results[c]["out"]).astype(np.float32)
        # [S, KC, 128, L] -> [S, C, L] -> [S, L, C]
        o = o.reshape(S, C, L).transpose(0, 2, 1)
        outs.append(o)
    out = np.concatenate(outs, axis=0)
    return np.ascontiguousarray(out.reshape(B, L, C)).astype(np.float32)


# revision 64
# speedup vs baseline: 4.4363x; 1.0137x over previous
"""Trainium2 Bass kernel for Convpass-swintransformer hypernet-mask adapter.

Data-parallel over batch: 8 NeuronCores x 8 samples each, all weights
replicated. All matmuls run in bf16 (fp32 PSUM accumulation); samples are
processed in pairs with block-diagonal weights so the 64-channel ops fill the
128-wide PE array. Biases are folded into activation/vector drains (or, for
the hypernet, a ones-row on the feat tile) instead of PE bias matmuls.
QuickGELU is a single Gelu_apprx_sigmoid activation; phase-B gelu ops are
gated behind phase A so the ACT function-table isn't thrashed against Exp.
"""
import sys

sys.path.insert(0, "/opt/trn_rl_repo")

import numpy as np

import concourse.bass as bass
import concourse.tile as tile
from concourse import bacc, mybir
from concourse.bass_utils import run_bass_kernel_spmd

AF = mybir.ActivationFunctionType
ALU = mybir.AluOpType
FP32 = mybir.dt.float32
BF16 = mybir.dt.bfloat16
BF16_NP = mybir.dt.np(BF16)

B, L, C = 64, 784, 384
DIM, NM, META = 64, 16, 64
HH, WW = 28, 28
NCORES = 8
S = B // NCORES          # samples per core
NPAIR = S // 2
KC = C // 128            # 3 contraction chunks for C=384
NPOS = [(0, 512), (512, 272)]   # 784 split at psum-bank boundary
NPOSB = [(0, 448), (448, 336)]  # 784 split matching the conv-psum halves
GROUPS = [(0, 3), (3, 3), (6, 2)]   # (first j8, n slots) per hypernet group

_CACHE = {}


def _build_nc():
    nc = bacc.Bacc(None)
    d = nc.declare_dram_parameter
    xt_d = d("xt", [NPAIR, 2, KC, 128, L], BF16, isOutput=False)
    cb_d = d("cb", [128, 2 * 384 + 64 + 32 + 128 + 384], BF16, isOutput=False)
    fb_d = d("fb", [128, 5], FP32, isOutput=False)
    ub_d = d("ub", [128, 3], FP32, isOutput=False)
    hw_d = d("hw", [65, 2, 9, 2048], BF16, isOutput=False)
    out_d = d("out", [S, KC, 128, L], BF16, isOutput=True)

    with tile.TileContext(nc) as tc:
        with tc.tile_pool(name="consts", bufs=1) as cp, \
             tc.tile_pool(name="hxp", bufs=2 * NPAIR) as hxp, \
             tc.tile_pool(name="padp", bufs=NPAIR) as padp, \
             tc.tile_pool(name="cwp", bufs=NPAIR) as cwpp, \
             tc.tile_pool(name="cwall", bufs=1) as cwp, \
             tc.tile_pool(name="hwp", bufs=9) as hwp:
            # ---- constants (two blobs) ----
            cb = cp.tile([128, 2 * 384 + 64 + 32 + 128 + 384], BF16)
            fb = cp.tile([128, 5], FP32)
            upb3 = cp.tile([128, 3], FP32)
            wab = [cb[:, 0:384].rearrange("p (k m) -> p k m", k=KC),
                   cb[:, 384:768].rearrange("p (k m) -> p k m", k=KC)]
            mw2dup = cb[:, 768:832]          # meta_w2 dup'd on both halves
            mtT2 = cb[:, 832:864]            # [128 n-pair, 32 m-pair]
            ones32 = cb[0:32, 864:992]
            upw = cb[:, 992:1376]            # upw dup'd on both halves
            b_a = [fb[:, 0:1], fb[:, 1:2]]   # per-sample-parity psa bias
            mb2p = fb[:, 2:3]
            bnd = [fb[:, 3:4], fb[:, 4:5]]   # relu bound: 0 on h-rows,
                                             # -3e38 on xd-rows, per parity

            feat_bf = cp.tile([65, 32], BF16)
            nc.vector.memset(feat_bf[:], 0.0)
            nc.vector.memset(feat_bf[64:65, :], 1.0)
            warm = cp.tile([64, 512], BF16)
            nc.vector.memset(warm[:], 0.0)
            with tc.tile_pool(name="psW", bufs=1, space="PSUM") as psW:
                psw = psW.tile([64, 512], FP32)
                for _ in range(3):
                    nc.tensor.matmul(psw[:], lhsT=warm[:, 0:64], rhs=warm[:],
                                     start=True, stop=True)
            featP = cp.tile([128, NPAIR], FP32)
            featQ = cp.tile([128, NPAIR], FP32)
            featPQ = [featP, featQ]
            zgate = cp.tile([128, 1], FP32)

            hx_tiles, pad_tiles, cw_tiles = [], [], []
            for s in range(S):
                hx = hxp.tile([128, L], BF16, name=f"hx{s}", tag="hx")
                hx_tiles.append(hx)
            for pr in range(NPAIR):
                pad = padp.tile([128, 900], BF16)
                nc.gpsimd.memset(pad[:], 0.0)
                pad_tiles.append(pad)
                cw = cwpp.tile([128, 9, 2, 64], BF16)
                nc.gpsimd.memset(cw[:], 0.0)
                cw_tiles.append(cw)

            # ================= phase A: meta-net / masks / feat =============
            # Software-pipelined: pair p+1's psa matmuls are interleaved
            # between pair p's small matmuls so the PE never drains, and the
            # ACT/DVE queues are emitted critical-path-first.
            with tc.tile_pool(name="xtp", bufs=4) as xtp, \
                 tc.tile_pool(name="psA", bufs=2, space="PSUM") as psA, \
                 tc.tile_pool(name="psB", bufs=4, space="PSUM") as psB, \
                 tc.tile_pool(name="sbA", bufs=2) as sbA, \
                 tc.tile_pool(name="smallA", bufs=2) as smA:
                xts = [None] * NPAIR
                prompts = [None] * NPAIR

                def emit_xt_dma(pr):
                    xt = xtp.tile([128, 2, KC, L], BF16, name=f"xt{pr}", tag="xt")
                    xts[pr] = xt
                    if pr == 0:
                        nc.sync.dma_start(   # first compute chunk leads
                            out=xt[:, 0, 0, :],
                            in_=xt_d[0, 0, 0].rearrange("p q -> p q"))
                        nc.sync.dma_start(out=cb[:], in_=cb_d[:])
                        for k in range(1, KC):
                            nc.sync.dma_start(
                                out=xt[:, 0, k, :],
                                in_=xt_d[0, 0, k].rearrange("p q -> p q"))
                        nc.sync.dma_start(
                            out=xt[:, 1, :, :],
                            in_=xt_d[0, 1].rearrange("k p q -> p k q"))
                        nc.sync.dma_start(out=fb[:], in_=fb_d[:])
                        nc.sync.dma_start(out=upb3[:], in_=ub_d[:])
                    else:
                        nc.sync.dma_start(
                            out=xt[:], in_=xt_d[pr].rearrange(
                                "s k p q -> p s k q"))

                def emit_psa(pr, h):
                    psa = psA.tile([128, L], FP32, tag="psa")
                    for n0, nw in NPOS:
                        for k in range(KC):
                            nc.tensor.matmul(
                                psa[:, n0:n0 + nw], lhsT=wab[h][:, k, :],
                                rhs=xts[pr][:, h, k, n0:n0 + nw],
                                start=(k == 0), stop=(k == KC - 1))
                    return psa

                def emit_hx(pr, h, psa):
                    # one fused drain per sample: (psa + bias) max bound,
                    # where bound is 0 on the h-rows (= relu) and -3e38 on
                    # the xd-rows (= identity). h-part at rows 64h, xd-part
                    # at rows 64*(1-h); phase B flips h to match.
                    nc.vector.tensor_scalar(
                        hx_tiles[2 * pr + h][:], psa[:],
                        b_a[h], bnd[h], ALU.add, ALU.max)

                # prologue: pair 0 psa + drains
                emit_xt_dma(0)
                for h in range(2):
                    psa = emit_psa(0, h)
                    emit_hx(0, h, psa)

                # All psB products split into 392-wide chunks: each chunk is
                # exactly one PSUM bank, so pool slots recycle at chunk
                # granularity and the psp(p+1) <- exp(p) coupling vanishes.
                CH = [(0, 392), (392, 392)]
                for pr in range(NPAIR):
                    nxt = pr + 1 if pr + 1 < NPAIR else None
                    prompt = sbA.tile([128, L], BF16, tag="prompt")
                    prompts[pr] = prompt
                    for n0, nw in CH:
                        psp = psB.tile([128, 392], FP32, tag="psb")
                        for h in range(2):
                            q0 = 64 * h     # h-part rows of sample 2pr+h
                            nc.tensor.matmul(
                                psp[q0:q0 + 64, :],
                                lhsT=mw2dup[q0:q0 + 64, :],
                                rhs=hx_tiles[2 * pr + h][q0:q0 + 64,
                                                         n0:n0 + nw],
                                start=True, stop=True)
                        nc.scalar.activation(prompt[:, n0:n0 + nw],
                                             psp[:], AF.Identity,
                                             bias=mb2p)

                    psa_n = [None, None]
                    if nxt is not None:
                        emit_xt_dma(nxt)
                        psa_n[0] = emit_psa(nxt, 0)

                    expt = sbA.tile([32, L], BF16, tag="expt")
                    zsum = [None, None]
                    for c, (n0, nw) in enumerate(CH):
                        psm = psB.tile([32, 392], FP32, tag="psb")
                        nc.tensor.matmul(psm[:], lhsT=mtT2,
                                         rhs=prompt[:, n0:n0 + nw],
                                         start=True, stop=True)
                        zsum[c] = smA.tile([32, 1], FP32, tag=f"z{c}",
                                           name=f"zs{pr}_{c}")
                        nc.scalar.activation(expt[:, n0:n0 + nw], psm[:],
                                             AF.Exp, accum_out=zsum[c][:])
                    invz = smA.tile([32, 1], FP32, tag="iz")
                    nc.vector.tensor_tensor(zsum[0][:], zsum[0][:],
                                            zsum[1][:], ALU.add)
                    nc.vector.reciprocal(invz[:], zsum[0][:])

                    if nxt is not None:
                        psa_n[1] = emit_psa(nxt, 1)

                    wones = sbA.tile([32, 128], BF16, tag="wones")
                    nc.vector.tensor_scalar_mul(wones[:], ones32, invz[:])
                    ftmp = sbA.tile([128, L], BF16, tag="ftmp")
                    for c, (n0, nw) in enumerate(CH):
                        pss = psB.tile([128, 392], FP32, tag="psb")
                        nc.tensor.matmul(pss[:], lhsT=wones[:],
                                         rhs=expt[:, n0:n0 + nw],
                                         start=True, stop=True)
                        nc.vector.tensor_mul(ftmp[:, n0:n0 + nw],
                                             pss[:],
                                             prompts[pr][:, n0:n0 + nw])
                        junk = sbA.tile([128, 392], BF16, tag="junk")
                        nc.scalar.activation(junk[:], ftmp[:, n0:n0 + nw],
                                             AF.Identity,
                                             accum_out=featPQ[c][:, pr:pr + 1])

                    if nxt is not None:
                        for h in range(2):
                            emit_hx(nxt, h, psa_n[h])

                    nc.vector.tensor_tensor(featP[:, pr:pr + 1],
                                            featP[:, pr:pr + 1],
                                            featQ[:, pr:pr + 1], ALU.add)
                    nc.vector.tensor_copy(feat_bf[0:64, 2 * pr:2 * pr + 1],
                                          featP[0:64, pr:pr + 1])
                    nc.vector.tensor_copy(feat_bf[0:64, 2 * pr + 1:2 * pr + 2],
                                          featP[64:128, pr:pr + 1])
                    if pr == NPAIR - 1:
                        # zero "gate": carries a dep on the last phase-A op so
                        # the scheduler can't hoist phase-B gelu (and its act
                        # table load) into phase A
                        nc.vector.tensor_scalar(
                            zgate[:], ftmp[:, 0:1], 0.0, None, ALU.mult)

            # ================= phase H: hypernet conv weights ===============
            # full-partition tiles: the strided-partition scatter reads then
            # stay inside one allocation for the access tracker
            cwalls = [cwp.tile([128, 9 * 512], BF16,
                               name=f"cwall{g}", tag=f"cwall{g}")
                      for g, (_, nb) in enumerate(GROUPS)]
            # pad-gelu for every pair only needs xd + the phase-A gate: emit
            # here so ACT runs them (and the gelu table load) during phase H
            for pr in range(NPAIR):
                pad3s = pad_tiles[pr].rearrange("p (r c) -> p r c", r=30)
                for h in range(2):
                    q0 = 64 - 64 * h    # xd rows of sample 2pr+h
                    nc.scalar.activation(
                        pad3s[q0:q0 + 64, 1:29, 1:29],
                        hx_tiles[2 * pr + h][q0:q0 + 64, :].rearrange(
                            "p (a b) -> p a b", a=28)[:],
                        AF.Gelu_apprx_sigmoid, bias=zgate[q0:q0 + 64])
            with tc.tile_pool(name="psH", bufs=6, space="PSUM") as psH:
                rot = 0
                for n9 in range(9):
                    hwc = hwp.tile([65, 2, 2048], BF16, tag="hw")
                    # Pool SWDGE queue: keeps these 9 transfers off the SP
                    # sequencer (~1.2us each there) and off the shared HWDGE
                    nc.gpsimd.dma_start(out=hwc[:], in_=hw_d[:, :, n9, :])
                    for g, (j8_0, nb) in enumerate(GROUPS):
                        psh = psH.tile([32 * nb, 512], FP32, tag="psh")
                        for a in range(nb):
                            h2, j4 = divmod(j8_0 + a, 4)
                            nc.tensor.matmul(
                                psh[32 * a:32 * a + 32, :],
                                lhsT=feat_bf[:, :],
                                rhs=hwc[:, h2, j4 * 512:(j4 + 1) * 512],
                                start=True, stop=True)
                        dst = cwalls[g][0:32 * nb, n9 * 512:(n9 + 1) * 512]
                        if rot == 0:
                            nc.vector.tensor_copy(dst, psh[:])
                        else:
                            nc.scalar.activation(dst, psh[:], AF.Copy)
                        rot = (rot + 1) % 2

            # pair-major, group-ascending inside a pair: the six scatters the
            # first conv needs come first, ordered to match drain completion.
            # Pairs 0-1 on the SP queue, pairs 2-3 on Pool SWDGE, so neither
            # queue's ~1.1-1.2us/DMA dispatch delays the first convs or the
            # output DMAs queued behind them.
            for pr in range(NPAIR):
                for g, (j8_0, nb) in enumerate(GROUPS):
                    for h in range(2):
                        s = 2 * pr + h
                        q = 1 - h       # storage half (matches xd layout)
                        eng = nc.sync if h == 0 else nc.gpsimd
                        eng.dma_start(
                            out=cw_tiles[pr][64 * q + 8 * j8_0:
                                             64 * q + 8 * (j8_0 + nb), :, q, :],
                            in_=cwalls[g][s:32 * nb:32].rearrange(
                                "p (i k o) -> p i k o", i=8, k=9))

            # ================= phase B: adapter conv + up ===================
            with tc.tile_pool(name="yap", bufs=2) as yap, \
                 tc.tile_pool(name="outp", bufs=3) as outp, \
                 tc.tile_pool(name="psC0", bufs=2, space="PSUM") as psC0, \
                 tc.tile_pool(name="psC1", bufs=2, space="PSUM") as psC1, \
                 tc.tile_pool(name="psU", bufs=2, space="PSUM") as psU:
                rot = 0
                convp = [None] * NPAIR

                def emit_conv(pr, taps):
                    pad3 = pad_tiles[pr].rearrange("p (r c) -> p r c", r=30)
                    if convp[pr] is None:
                        convp[pr] = (psC0.tile([128, 448], FP32, name=f"c0_{pr}", tag="c0"),
                                     psC1.tile([128, 336], FP32, name=f"c1_{pr}", tag="c1"))
                    ps0, ps1 = convp[pr]
                    for k9 in taps:
                        ky, kx = divmod(k9, 3)
                        lw = cw_tiles[pr][:, k9, :, :]
                        nc.tensor.matmul(
                            ps0[:], lhsT=lw,
                            rhs=pad3[:, ky:ky + 16, kx:kx + 28],
                            start=(k9 == 0), stop=(k9 == 8))
                        nc.tensor.matmul(
                            ps1[:], lhsT=lw,
                            rhs=pad3[:, ky + 16:ky + 28, kx:kx + 28],
                            start=(k9 == 0), stop=(k9 == 8))

                def emit_up(pr):
                    nonlocal rot
                    ps0, ps1 = convp[pr]
                    ya = yap.tile([128, L], BF16, tag="ya")
                    nc.scalar.activation(ya[:, 0:448], ps0[:],
                                         AF.Gelu_apprx_sigmoid)
                    nc.scalar.activation(ya[:, 448:784], ps1[:],
                                         AF.Gelu_apprx_sigmoid)
                    last = pr == NPAIR - 1
                    for q in range(2):
                        h = 1 - q       # sample parity stored in half q
                        outt = outp.tile([128, KC, L], BF16, tag="outt")
                        for j3 in range(KC):
                            # one 2-bank psum tile per (h, j3); the two
                            # matmuls split at the bank boundary, one drain
                            psu = psU.tile([128, L], FP32, tag="psu")
                            for n0, nw in NPOS:
                                nc.tensor.matmul(
                                    psu[:, n0:n0 + nw],
                                    lhsT=upw[64 * q:64 * q + 64,
                                             128 * j3:128 * (j3 + 1)],
                                    rhs=ya[64 * q:64 * q + 64, n0:n0 + nw],
                                    start=True, stop=True)
                            dst = outt[:, j3, :]
                            on_dve = (rot % 2 == 0) if last else (rot % 3 != 2)
                            if on_dve:
                                nc.vector.tensor_scalar(
                                    dst, psu[:], upb3[:, j3:j3 + 1],
                                    None, ALU.add)
                            else:
                                nc.scalar.activation(
                                    dst, psu[:], AF.Identity,
                                    bias=upb3[:, j3:j3 + 1])
                            rot += 1
                            if last and q == 0:
                                # chunked final DMA shortens the drain tail
                                nc.sync.dma_start(
                                    out=out_d[2 * pr + h, j3].rearrange(
                                        "p q -> p q"),
                                    in_=outt[:, j3, :])
                        if not (last and q == 0):
                            nc.sync.dma_start(
                                out=out_d[2 * pr + h].rearrange(
                                    "k p q -> p k q"),
                                in_=outt[:])

                # software pipeline: up(p)'s qgelu latency is covered by the
                # first taps of conv(p+1); its drain tail by the rest
                emit_conv(0, range(9))
                for pr in range(1, NPAIR):
                    emit_conv(pr, range(0, 3))
                    emit_up(pr - 1)
                    emit_conv(pr, range(3, 9))
                emit_up(NPAIR - 1)
    nc.finalize()
    return nc


def _prep(x, meta_w1, meta_b1, meta_w2, meta_b2, mask_token,
          hyper_w, hyper_b, down_w, down_b, up_w, up_b):
    f = lambda a: np.ascontiguousarray(np.asarray(a, dtype=np.float32))
    bf = lambda a: np.ascontiguousarray(np.asarray(a).astype(BF16_NP))
    x = f(x)
    xt = x.reshape(B, L, C).transpose(0, 2, 1)            # [B, C, L]
    xt = bf(xt).reshape(B, KC, 128, L).reshape(NCORES, NPAIR, 2, KC, 128, L)

    # psa weights: even parity [meta_w1 | down_w], odd parity swapped
    wA0 = np.concatenate([f(meta_w1), f(down_w)], axis=1)   # [384, 128]
    wA1 = np.concatenate([f(down_w), f(meta_w1)], axis=1)
    wab = np.stack([wA0, wA1]).reshape(2, KC, 128, 128).transpose(
        0, 2, 1, 3).reshape(2, 128, 384)                    # [par, p, (k m)]

    mtT2s = np.zeros((32, 128), np.float32)     # [m-pair, n-pair]
    mtT2s[0:16, 0:64] = f(mask_token)
    mtT2s[16:32, 64:128] = f(mask_token)
    ones32 = np.zeros((32, 128), np.float32)
    ones32[0:16, 0:64] = 1.0
    ones32[16:32, 64:128] = 1.0
    upw = f(up_w)                                # [64, 384]

    cbw = 2 * 384 + 64 + 32 + 128 + 384
    cb = np.zeros((128, cbw), np.float32)
    cb[:, 0:384] = wab[0]
    cb[:, 384:768] = wab[1]
    cb[0:64, 768:832] = f(meta_w2)
    cb[64:128, 768:832] = f(meta_w2)            # dup for base-partition match
    cb[:, 832:864] = mtT2s.T                    # [128 n-pair, 32 m-pair]
    cb[0:32, 864:992] = ones32
    cb[0:64, 992:1376] = upw
    cb[64:128, 992:1376] = upw                  # dup for base-partition match
    cb = bf(cb)

    fbm = np.zeros((128, 5), np.float32)
    fbm[0:64, 0] = f(meta_b1)
    fbm[64:128, 0] = f(down_b)
    fbm[0:64, 1] = f(down_b)
    fbm[64:128, 1] = f(meta_b1)
    fbm[0:64, 2] = f(meta_b2)
    fbm[64:128, 2] = f(meta_b2)
    fbm[0:64, 3] = 0.0                          # relu bound, parity 0
    fbm[64:128, 3] = -3.0e38
    fbm[0:64, 4] = -3.0e38                      # relu bound, parity 1
    fbm[64:128, 4] = 0.0

    upb3 = f(up_b).reshape(KC, 128).T            # [128, 3]

    # hypernet weights: columns packed (j8, i8, ky, kx, o); ones-row = hyper_b
    hw5 = f(hyper_w).reshape(META, DIM, DIM, 3, 3)       # [n, o, i, ky, kx]
    hwc = hw5.transpose(0, 2, 3, 4, 1).reshape(META, 8, 4608)  # [n, j8, (i8 k o)]
    hwc = hwc.reshape(META, 2, 4, 9, 512).transpose(0, 1, 3, 2, 4).reshape(
        META, 2, 9, 2048)
    hb5 = f(hyper_b).reshape(DIM, DIM, 3, 3)             # [o, i, ky, kx]
    hbc = hb5.transpose(1, 2, 3, 0).reshape(8, 4608)     # [j8, (i8 k o)]
    hbc = hbc.reshape(2, 4, 9, 512).transpose(0, 2, 1, 3).reshape(2, 9, 2048)
    hwe = np.concatenate([hwc, hbc[None]], axis=0)       # [65, 2, 9, 2048]
    hwe = bf(hwe)

    consts = {"cb": cb, "fb": fbm, "hw": hwe, "ub": np.ascontiguousarray(upb3)}
    in_maps = []
    for c in range(NCORES):
        m = dict(consts)
        m["xt"] = np.ascontiguousarray(xt[c])
        in_maps.append(m)
    return in_maps


def _run(in_maps, **kw):
    if "nc" not in _CACHE:
        _CACHE["nc"] = _build_nc()
    return run_bass_kernel_spmd(_CACHE["nc"], in_maps, list(range(NCORES)), **kw)


def kernel(x, meta_w1, meta_b1, meta_w2, meta_b2, mask_token,
           hyper_w, hyper_b, down_w, down_b, up_w, up_b, H, W):
    assert int(H) == HH and int(W) == WW
    in_maps = _prep(x, meta_w1, meta_b1, meta_w2, meta_b2, mask_token,
                    hyper_w, hyper_b, down_w, down_b, up_w, up_b)
    res = _run(in_maps)
    outs = []
    for c in range(NCORES):
        o = np.asarray(res.results[c]["out"]).astype(np.float32)
        # [S, KC, 128, L] -> [S, C, L] -> [S, L, C]
        o = o.reshape(S, C, L).transpose(0, 2, 1)
        outs.append(o)
    out = np.concatenate(outs, axis=0)
    return np.ascontiguousarray(out.reshape(B, L, C)).astype(np.float32)


# revision 67
# speedup vs baseline: 4.4445x; 1.0019x over previous
"""Trainium2 Bass kernel for Convpass-swintransformer hypernet-mask adapter.

Data-parallel over batch: 8 NeuronCores x 8 samples each, all weights
replicated. All matmuls run in bf16 (fp32 PSUM accumulation); samples are
processed in pairs with block-diagonal weights so the 64-channel ops fill the
128-wide PE array. Biases are folded into activation/vector drains (or, for
the hypernet, a ones-row on the feat tile) instead of PE bias matmuls.
QuickGELU is a single Gelu_apprx_sigmoid activation; phase-B gelu ops are
gated behind phase A so the ACT function-table isn't thrashed against Exp.
"""
import sys

sys.path.insert(0, "/opt/trn_rl_repo")

import numpy as np

import concourse.bass as bass
import concourse.tile as tile
from concourse import bacc, mybir
from concourse.bass_utils import run_bass_kernel_spmd

AF = mybir.ActivationFunctionType
ALU = mybir.AluOpType
FP32 = mybir.dt.float32
BF16 = mybir.dt.bfloat16
BF16_NP = mybir.dt.np(BF16)

B, L, C = 64, 784, 384
DIM, NM, META = 64, 16, 64
HH, WW = 28, 28
NCORES = 8
S = B // NCORES          # samples per core
NPAIR = S // 2
KC = C // 128            # 3 contraction chunks for C=384
NPOS = [(0, 512), (512, 272)]   # 784 split at psum-bank boundary
NPOSB = [(0, 448), (448, 336)]  # 784 split matching the conv-psum halves
GROUPS = [(0, 3), (3, 3), (6, 2)]   # (first j8, n slots) per hypernet group

_CACHE = {}


def _build_nc():
    nc = bacc.Bacc(None)
    d = nc.declare_dram_parameter
    xt_d = d("xt", [NPAIR, 2, KC, 128, L], BF16, isOutput=False)
    cb_d = d("cb", [128, 2 * 384 + 64 + 32 + 128 + 384], BF16, isOutput=False)
    fb_d = d("fb", [128, 5], FP32, isOutput=False)
    ub_d = d("ub", [128, 3], FP32, isOutput=False)
    hw_d = d("hw", [65, 2, 9, 2048], BF16, isOutput=False)
    out_d = d("out", [S, KC, 128, L], BF16, isOutput=True)

    with tile.TileContext(nc) as tc:
        with tc.tile_pool(name="consts", bufs=1) as cp, \
             tc.tile_pool(name="hxp", bufs=2 * NPAIR) as hxp, \
             tc.tile_pool(name="padp", bufs=NPAIR) as padp, \
             tc.tile_pool(name="cwp", bufs=NPAIR) as cwpp, \
             tc.tile_pool(name="cwall", bufs=1) as cwp, \
             tc.tile_pool(name="hwp", bufs=9) as hwp:
            # ---- constants (two blobs) ----
            cb = cp.tile([128, 2 * 384 + 64 + 32 + 128 + 384], BF16)
            fb = cp.tile([128, 5], FP32)
            upb3 = cp.tile([128, 3], FP32)
            wab = [cb[:, 0:384].rearrange("p (k m) -> p k m", k=KC),
                   cb[:, 384:768].rearrange("p (k m) -> p k m", k=KC)]
            mw2dup = cb[:, 768:832]          # meta_w2 dup'd on both halves
            mtT2 = cb[:, 832:864]            # [128 n-pair, 32 m-pair]
            ones32 = cb[0:32, 864:992]
            upw = cb[:, 992:1376]            # upw dup'd on both halves
            b_a = [fb[:, 0:1], fb[:, 1:2]]   # per-sample-parity psa bias
            mb2p = fb[:, 2:3]
            bnd = [fb[:, 3:4], fb[:, 4:5]]   # relu bound: 0 on h-rows,
                                             # -3e38 on xd-rows, per parity

            feat_bf = cp.tile([65, 32], BF16)
            nc.vector.memset(feat_bf[:], 0.0)
            nc.vector.memset(feat_bf[64:65, :], 1.0)
            warm = cp.tile([64, 512], BF16)
            nc.vector.memset(warm[:], 0.0)
            with tc.tile_pool(name="psW", bufs=1, space="PSUM") as psW:
                psw = psW.tile([64, 512], FP32)
                for _ in range(3):
                    nc.tensor.matmul(psw[:], lhsT=warm[:, 0:64], rhs=warm[:],
                                     start=True, stop=True)
            featP = cp.tile([128, NPAIR], FP32)
            featQ = cp.tile([128, NPAIR], FP32)
            featPQ = [featP, featQ]
            zgate = cp.tile([128, 1], FP32)

            hx_tiles, pad_tiles, cw_tiles = [], [], []
            for s in range(S):
                hx = hxp.tile([128, L], BF16, name=f"hx{s}", tag="hx")
                hx_tiles.append(hx)
            for pr in range(NPAIR):
                pad = padp.tile([128, 900], BF16)
                nc.gpsimd.memset(pad[:], 0.0)
                pad_tiles.append(pad)
                cw = cwpp.tile([128, 9, 2, 64], BF16)
                nc.gpsimd.memset(cw[:], 0.0)
                cw_tiles.append(cw)

            # ================= phase A: meta-net / masks / feat =============
            # Software-pipelined: pair p+1's psa matmuls are interleaved
            # between pair p's small matmuls so the PE never drains, and the
            # ACT/DVE queues are emitted critical-path-first.
            with tc.tile_pool(name="xtp", bufs=4) as xtp, \
                 tc.tile_pool(name="psA", bufs=2, space="PSUM") as psA, \
                 tc.tile_pool(name="psB", bufs=4, space="PSUM") as psB, \
                 tc.tile_pool(name="sbA", bufs=3) as sbA, \
                 tc.tile_pool(name="smallA", bufs=4) as smA:
                xts = [None] * NPAIR
                prompts = [None] * NPAIR

                def emit_xt_dma(pr):
                    xt = xtp.tile([128, 2, KC, L], BF16, name=f"xt{pr}", tag="xt")
                    xts[pr] = xt
                    if pr == 0:
                        nc.sync.dma_start(   # first compute chunk leads
                            out=xt[:, 0, 0, :],
                            in_=xt_d[0, 0, 0].rearrange("p q -> p q"))
                        nc.sync.dma_start(out=cb[:], in_=cb_d[:])
                        for k in range(1, KC):
                            nc.sync.dma_start(
                                out=xt[:, 0, k, :],
                                in_=xt_d[0, 0, k].rearrange("p q -> p q"))
                        nc.sync.dma_start(
                            out=xt[:, 1, :, :],
                            in_=xt_d[0, 1].rearrange("k p q -> p k q"))
                        nc.sync.dma_start(out=fb[:], in_=fb_d[:])
                        nc.sync.dma_start(out=upb3[:], in_=ub_d[:])
                    else:
                        nc.sync.dma_start(
                            out=xt[:], in_=xt_d[pr].rearrange(
                                "s k p q -> p s k q"))

                def emit_psa(pr, h):
                    psa = psA.tile([128, L], FP32, tag="psa")
                    for n0, nw in NPOS:
                        for k in range(KC):
                            nc.tensor.matmul(
                                psa[:, n0:n0 + nw], lhsT=wab[h][:, k, :],
                                rhs=xts[pr][:, h, k, n0:n0 + nw],
                                start=(k == 0), stop=(k == KC - 1))
                    return psa

                def emit_hx(pr, h, psa):
                    # one fused drain per sample: (psa + bias) max bound,
                    # where bound is 0 on the h-rows (= relu) and -3e38 on
                    # the xd-rows (= identity). h-part at rows 64h, xd-part
                    # at rows 64*(1-h); phase B flips h to match.
                    nc.vector.tensor_scalar(
                        hx_tiles[2 * pr + h][:], psa[:],
                        b_a[h], bnd[h], ALU.add, ALU.max)

                # prologue: pair 0 psa + drains
                emit_xt_dma(0)
                for h in range(2):
                    psa = emit_psa(0, h)
                    emit_hx(0, h, psa)

                # All psB products split into 392-wide chunks: each chunk is
                # exactly one PSUM bank, so pool slots recycle at chunk
                # granularity and the psp(p+1) <- exp(p) coupling vanishes.
                CH = [(0, 512), (512, 272)]
                for pr in range(NPAIR):
                    nxt = pr + 1 if pr + 1 < NPAIR else None
                    prompt = sbA.tile([128, L], BF16, tag="prompt")
                    prompts[pr] = prompt
                    for n0, nw in CH:
                        psp = psB.tile([128, nw], FP32, tag="psb")
                        for h in range(2):
                            q0 = 64 * h     # h-part rows of sample 2pr+h
                            nc.tensor.matmul(
                                psp[q0:q0 + 64, :],
                                lhsT=mw2dup[q0:q0 + 64, :],
                                rhs=hx_tiles[2 * pr + h][q0:q0 + 64,
                                                         n0:n0 + nw],
                                start=True, stop=True)
                        nc.scalar.activation(prompt[:, n0:n0 + nw],
                                             psp[:], AF.Identity,
                                             bias=mb2p)

                    psa_n = [None, None]
                    if nxt is not None:
                        emit_xt_dma(nxt)
                        psa_n[0] = emit_psa(nxt, 0)

                    expt = sbA.tile([32, L], BF16, tag="expt")
                    zsum = [None, None]
                    for c, (n0, nw) in enumerate(CH):
                        psm = psB.tile([32, nw], FP32, tag="psb")
                        nc.tensor.matmul(psm[:], lhsT=mtT2,
                                         rhs=prompt[:, n0:n0 + nw],
                                         start=True, stop=True)
                        zsum[c] = smA.tile([32, 1], FP32, tag=f"z{c}",
                                           name=f"zs{pr}_{c}")
                        nc.scalar.activation(expt[:, n0:n0 + nw], psm[:],
                                             AF.Exp, accum_out=zsum[c][:])
                    invz = smA.tile([32, 1], FP32, tag="iz")
                    nc.vector.tensor_tensor(zsum[0][:], zsum[0][:],
                                            zsum[1][:], ALU.add)
                    nc.vector.reciprocal(invz[:], zsum[0][:])

                    if nxt is not None:
                        psa_n[1] = emit_psa(nxt, 1)

                    wones = sbA.tile([32, 128], BF16, tag="wones")
                    nc.vector.tensor_scalar_mul(wones[:], ones32, invz[:])
                    ftmp = sbA.tile([128, L], BF16, tag="ftmp")
                    for c, (n0, nw) in enumerate(CH):
                        pss = psB.tile([128, nw], FP32, tag="psb")
                        nc.tensor.matmul(pss[:], lhsT=wones[:],
                                         rhs=expt[:, n0:n0 + nw],
                                         start=True, stop=True)
                        nc.vector.tensor_mul(ftmp[:, n0:n0 + nw],
                                             pss[:],
                                             prompts[pr][:, n0:n0 + nw])
                        junk = sbA.tile([128, 392], BF16, tag="junk")
                        nc.scalar.activation(junk[:], ftmp[:, n0:n0 + nw],
                                             AF.Identity,
                                             accum_out=featPQ[c][:, pr:pr + 1])

                    if nxt is not None:
                        for h in range(2):
                            emit_hx(nxt, h, psa_n[h])

                    nc.vector.tensor_tensor(featP[:, pr:pr + 1],
                                            featP[:, pr:pr + 1],
                                            featQ[:, pr:pr + 1], ALU.add)
                    nc.vector.tensor_copy(feat_bf[0:64, 2 * pr:2 * pr + 1],
                                          featP[0:64, pr:pr + 1])
                    nc.vector.tensor_copy(feat_bf[0:64, 2 * pr + 1:2 * pr + 2],
                                          featP[64:128, pr:pr + 1])
                    if pr == NPAIR - 1:
                        # zero "gate": carries a dep on the last phase-A op so
                        # the scheduler can't hoist phase-B gelu (and its act
                        # table load) into phase A
                        nc.vector.tensor_scalar(
                            zgate[:], ftmp[:, 0:1], 0.0, None, ALU.mult)

            # ================= phase H: hypernet conv weights ===============
            # full-partition tiles: the strided-partition scatter reads then
            # stay inside one allocation for the access tracker
            cwalls = [cwp.tile([128, 9 * 512], BF16,
                               name=f"cwall{g}", tag=f"cwall{g}")
                      for g, (_, nb) in enumerate(GROUPS)]
            # pad-gelu for every pair only needs xd + the phase-A gate: emit
            # here so ACT runs them (and the gelu table load) during phase H
            for pr in range(NPAIR):
                pad3s = pad_tiles[pr].rearrange("p (r c) -> p r c", r=30)
                for h in range(2):
                    q0 = 64 - 64 * h    # xd rows of sample 2pr+h
                    nc.scalar.activation(
                        pad3s[q0:q0 + 64, 1:29, 1:29],
                        hx_tiles[2 * pr + h][q0:q0 + 64, :].rearrange(
                            "p (a b) -> p a b", a=28)[:],
                        AF.Gelu_apprx_sigmoid, bias=zgate[q0:q0 + 64])
            with tc.tile_pool(name="psH", bufs=6, space="PSUM") as psH:
                rot = 0
                for n9 in range(9):
                    hwc = hwp.tile([65, 2, 2048], BF16, tag="hw")
                    # Pool SWDGE queue: keeps these 9 transfers off the SP
                    # sequencer (~1.2us each there) and off the shared HWDGE
                    nc.gpsimd.dma_start(out=hwc[:], in_=hw_d[:, :, n9, :])
                    for g, (j8_0, nb) in enumerate(GROUPS):
                        psh = psH.tile([32 * nb, 512], FP32, tag="psh")
                        for a in range(nb):
                            h2, j4 = divmod(j8_0 + a, 4)
                            nc.tensor.matmul(
                                psh[32 * a:32 * a + 32, :],
                                lhsT=feat_bf[:, :],
                                rhs=hwc[:, h2, j4 * 512:(j4 + 1) * 512],
                                start=True, stop=True)
                        dst = cwalls[g][0:32 * nb, n9 * 512:(n9 + 1) * 512]
                        if rot == 0:
                            nc.vector.tensor_copy(dst, psh[:])
                        else:
                            nc.scalar.activation(dst, psh[:], AF.Copy)
                        rot = (rot + 1) % 2

            # pair-major, group-ascending inside a pair: the six scatters the
            # first conv needs come first, ordered to match drain completion.
            # Pairs 0-1 on the SP queue, pairs 2-3 on Pool SWDGE, so neither
            # queue's ~1.1-1.2us/DMA dispatch delays the first convs or the
            # output DMAs queued behind them.
            for pr in range(NPAIR):
                for g, (j8_0, nb) in enumerate(GROUPS):
                    for h in range(2):
                        s = 2 * pr + h
                        q = 1 - h       # storage half (matches xd layout)
                        eng = nc.sync if h == 0 else nc.gpsimd
                        eng.dma_start(
                            out=cw_tiles[pr][64 * q + 8 * j8_0:
                                             64 * q + 8 * (j8_0 + nb), :, q, :],
                            in_=cwalls[g][s:32 * nb:32].rearrange(
                                "p (i k o) -> p i k o", i=8, k=9))

            # ================= phase B: adapter conv + up ===================
            with tc.tile_pool(name="yap", bufs=2) as yap, \
                 tc.tile_pool(name="outp", bufs=3) as outp, \
                 tc.tile_pool(name="psC0", bufs=2, space="PSUM") as psC0, \
                 tc.tile_pool(name="psC1", bufs=2, space="PSUM") as psC1, \
                 tc.tile_pool(name="psU", bufs=2, space="PSUM") as psU:
                rot = 0
                convp = [None] * NPAIR

                def emit_conv(pr, taps):
                    pad3 = pad_tiles[pr].rearrange("p (r c) -> p r c", r=30)
                    if convp[pr] is None:
                        convp[pr] = (psC0.tile([128, 448], FP32, name=f"c0_{pr}", tag="c0"),
                                     psC1.tile([128, 336], FP32, name=f"c1_{pr}", tag="c1"))
                    ps0, ps1 = convp[pr]
                    for k9 in taps:
                        ky, kx = divmod(k9, 3)
                        lw = cw_tiles[pr][:, k9, :, :]
                        nc.tensor.matmul(
                            ps0[:], lhsT=lw,
                            rhs=pad3[:, ky:ky + 16, kx:kx + 28],
                            start=(k9 == 0), stop=(k9 == 8))
                        nc.tensor.matmul(
                            ps1[:], lhsT=lw,
                            rhs=pad3[:, ky + 16:ky + 28, kx:kx + 28],
                            start=(k9 == 0), stop=(k9 == 8))

                def emit_up(pr):
                    nonlocal rot
                    ps0, ps1 = convp[pr]
                    ya = yap.tile([128, L], BF16, tag="ya")
                    nc.scalar.activation(ya[:, 0:448], ps0[:],
                                         AF.Gelu_apprx_sigmoid)
                    nc.scalar.activation(ya[:, 448:784], ps1[:],
                                         AF.Gelu_apprx_sigmoid)
                    last = pr == NPAIR - 1
                    for q in range(2):
                        h = 1 - q       # sample parity stored in half q
                        outt = outp.tile([128, KC, L], BF16, tag="outt")
                        for j3 in range(KC):
                            # one 2-bank psum tile per (h, j3); the two
                            # matmuls split at the bank boundary, one drain
                            psu = psU.tile([128, L], FP32, tag="psu")
                            for n0, nw in NPOS:
                                nc.tensor.matmul(
                                    psu[:, n0:n0 + nw],
                                    lhsT=upw[64 * q:64 * q + 64,
                                             128 * j3:128 * (j3 + 1)],
                                    rhs=ya[64 * q:64 * q + 64, n0:n0 + nw],
                                    start=True, stop=True)
                            dst = outt[:, j3, :]
                            on_dve = (rot % 2 == 0) if last else (rot % 3 != 2)
                            if on_dve:
                                nc.vector.tensor_scalar(
                                    dst, psu[:], upb3[:, j3:j3 + 1],
                                    None, ALU.add)
                            else:
                                nc.scalar.activation(
                                    dst, psu[:], AF.Identity,
                                    bias=upb3[:, j3:j3 + 1])
                            rot += 1
                            if last and q == 0:
                                # chunked final DMA shortens the drain tail
                                nc.sync.dma_start(
                                    out=out_d[2 * pr + h, j3].rearrange(
                                        "p q -> p q"),
                                    in_=outt[:, j3, :])
                        if not (last and q == 0):
                            nc.sync.dma_start(
                                out=out_d[2 * pr + h].rearrange(
                                    "k p q -> p k q"),
                                in_=outt[:])

                # software pipeline: up(p)'s qgelu latency is covered by the
                # first taps of conv(p+1); its drain tail by the rest
                emit_conv(0, range(9))
                for pr in range(1, NPAIR):
                    emit_conv(pr, range(0, 3))
                    emit_up(pr - 1)
                    emit_conv(pr, range(3, 9))
                emit_up(NPAIR - 1)
    nc.finalize()
    return nc


def _prep(x, meta_w1, meta_b1, meta_w2, meta_b2, mask_token,
          hyper_w, hyper_b, down_w, down_b, up_w, up_b):
    f = lambda a: np.ascontiguousarray(np.asarray(a, dtype=np.float32))
    bf = lambda a: np.ascontiguousarray(np.asarray(a).astype(BF16_NP))
    x = f(x)
    xt = x.reshape(B, L, C).transpose(0, 2, 1)            # [B, C, L]
    xt = bf(xt).reshape(B, KC, 128, L).reshape(NCORES, NPAIR, 2, KC, 128, L)

    # psa weights: even parity [meta_w1 | down_w], odd parity swapped
    wA0 = np.concatenate([f(meta_w1), f(down_w)], axis=1)   # [384, 128]
    wA1 = np.concatenate([f(down_w), f(meta_w1)], axis=1)
    wab = np.stack([wA0, wA1]).reshape(2, KC, 128, 128).transpose(
        0, 2, 1, 3).reshape(2, 128, 384)                    # [par, p, (k m)]

    mtT2s = np.zeros((32, 128), np.float32)     # [m-pair, n-pair]
    mtT2s[0:16, 0:64] = f(mask_token)
    mtT2s[16:32, 64:128] = f(mask_token)
    ones32 = np.zeros((32, 128), np.float32)
    ones32[0:16, 0:64] = 1.0
    ones32[16:32, 64:128] = 1.0
    upw = f(up_w)                                # [64, 384]

    cbw = 2 * 384 + 64 + 32 + 128 + 384
    cb = np.zeros((128, cbw), np.float32)
    cb[:, 0:384] = wab[0]
    cb[:, 384:768] = wab[1]
    cb[0:64, 768:832] = f(meta_w2)
    cb[64:128, 768:832] = f(meta_w2)            # dup for base-partition match
    cb[:, 832:864] = mtT2s.T                    # [128 n-pair, 32 m-pair]
    cb[0:32, 864:992] = ones32
    cb[0:64, 992:1376] = upw
    cb[64:128, 992:1376] = upw                  # dup for base-partition match
    cb = bf(cb)

    fbm = np.zeros((128, 5), np.float32)
    fbm[0:64, 0] = f(meta_b1)
    fbm[64:128, 0] = f(down_b)
    fbm[0:64, 1] = f(down_b)
    fbm[64:128, 1] = f(meta_b1)
    fbm[0:64, 2] = f(meta_b2)
    fbm[64:128, 2] = f(meta_b2)
    fbm[0:64, 3] = 0.0                          # relu bound, parity 0
    fbm[64:128, 3] = -3.0e38
    fbm[0:64, 4] = -3.0e38                      # relu bound, parity 1
    fbm[64:128, 4] = 0.0

    upb3 = f(up_b).reshape(KC, 128).T            # [128, 3]

    # hypernet weights: columns packed (j8, i8, ky, kx, o); ones-row = hyper_b
    hw5 = f(hyper_w).reshape(META, DIM, DIM, 3, 3)       # [n, o, i, ky, kx]
    hwc = hw5.transpose(0, 2, 3, 4, 1).reshape(META, 8, 4608)  # [n, j8, (i8 k o)]
    hwc = hwc.reshape(META, 2, 4, 9, 512).transpose(0, 1, 3, 2, 4).reshape(
        META, 2, 9, 2048)
    hb5 = f(hyper_b).reshape(DIM, DIM, 3, 3)             # [o, i, ky, kx]
    hbc = hb5.transpose(1, 2, 3, 0).reshape(8, 4608)     # [j8, (i8 k o)]
    hbc = hbc.reshape(2, 4, 9, 512).transpose(0, 2, 1, 3).reshape(2, 9, 2048)
    hwe = np.concatenate([hwc, hbc[None]], axis=0)       # [65, 2, 9, 2048]
    hwe = bf(hwe)

    consts = {"cb": cb, "fb": fbm, "hw": hwe, "ub": np.ascontiguousarray(upb3)}
    in_maps = []
    for c in range(NCORES):
        m = dict(consts)
        m["xt"] = np.ascontiguousarray(xt[c])
        in_maps.append(m)
    return in_maps


def _run(in_maps, **kw):
    if "nc" not in _CACHE:
        _CACHE["nc"] = _build_nc()
    return run_bass_kernel_spmd(_CACHE["nc"], in_maps, list(range(NCORES)), **kw)


def kernel(x, meta_w1, meta_b1, meta_w2, meta_b2, mask_token,
           hyper_w, hyper_b, down_w, down_b, up_w, up_b, H, W):
    assert int(H) == HH and int(W) == WW
    in_maps = _prep(x, meta_w1, meta_b1, meta_w2, meta_b2, mask_token,
                    hyper_w, hyper_b, down_w, down_b, up_w, up_b)
    res = _run(in_maps)
    outs = []
    for c in range(NCORES):
        o = np.asarray(res.results[c]["out"]).astype(np.float32)
        # [S, KC, 128, L] -> [S, C, L] -> [S, L, C]
        o = o.reshape(S, C, L).transpose(0, 2, 1)
        outs.append(o)
    out = np.concatenate(outs, axis=0)
    return np.ascontiguousarray(out.reshape(B, L, C)).astype(np.float32)


# revision 73
# speedup vs baseline: 4.4583x; 1.0031x over previous
"""Trainium2 Bass kernel for Convpass-swintransformer hypernet-mask adapter.

Data-parallel over batch: 8 NeuronCores x 8 samples each, all weights
replicated. All matmuls run in bf16 (fp32 PSUM accumulation); samples are
processed in pairs with block-diagonal weights so the 64-channel ops fill the
128-wide PE array. Biases are folded into activation/vector drains (or, for
the hypernet, a ones-row on the feat tile) instead of PE bias matmuls.
QuickGELU is a single Gelu_apprx_sigmoid activation; phase-B gelu ops are
gated behind phase A so the ACT function-table isn't thrashed against Exp.
"""
import sys

sys.path.insert(0, "/opt/trn_rl_repo")

import numpy as np

import concourse.bass as bass
import concourse.tile as tile
from concourse import bacc, mybir
from concourse.bass_utils import run_bass_kernel_spmd

AF = mybir.ActivationFunctionType
ALU = mybir.AluOpType
FP32 = mybir.dt.float32
BF16 = mybir.dt.bfloat16
BF16_NP = mybir.dt.np(BF16)

B, L, C = 64, 784, 384
DIM, NM, META = 64, 16, 64
HH, WW = 28, 28
NCORES = 8
S = B // NCORES          # samples per core
NPAIR = S // 2
KC = C // 128            # 3 contraction chunks for C=384
NPOS = [(0, 512), (512, 272)]   # 784 split at psum-bank boundary
NPOSB = [(0, 448), (448, 336)]  # 784 split matching the conv-psum halves
GROUPS = [(0, 3), (3, 3), (6, 2)]   # (first j8, n slots) per hypernet group

_CACHE = {}


def _build_nc():
    nc = bacc.Bacc(None)
    d = nc.declare_dram_parameter
    xt_d = d("xt", [NPAIR, 2, KC, 128, L], BF16, isOutput=False)
    cb_d = d("cb", [128, 2 * 384 + 64 + 32 + 128 + 384], BF16, isOutput=False)
    fb_d = d("fb", [128, 5], FP32, isOutput=False)
    ub_d = d("ub", [128, 3], FP32, isOutput=False)
    hw_d = d("hw", [65, 2, 9, 2048], BF16, isOutput=False)
    out_d = d("out", [S, KC, 128, L], BF16, isOutput=True)

    with tile.TileContext(nc) as tc:
        with tc.tile_pool(name="consts", bufs=1) as cp, \
             tc.tile_pool(name="hxp", bufs=2 * NPAIR) as hxp, \
             tc.tile_pool(name="padp", bufs=NPAIR) as padp, \
             tc.tile_pool(name="cwp", bufs=NPAIR) as cwpp, \
             tc.tile_pool(name="cwall", bufs=1) as cwp, \
             tc.tile_pool(name="hwp", bufs=9) as hwp:
            # ---- constants (two blobs) ----
            cb = cp.tile([128, 2 * 384 + 64 + 32 + 128 + 384], BF16)
            fb = cp.tile([128, 5], FP32)
            upb3 = cp.tile([128, 3], FP32)
            wab = [cb[:, 0:384].rearrange("p (k m) -> p k m", k=KC),
                   cb[:, 384:768].rearrange("p (k m) -> p k m", k=KC)]
            mw2dup = cb[:, 768:832]          # meta_w2 dup'd on both halves
            mtT2 = cb[:, 832:864]            # [128 n-pair, 32 m-pair]
            ones32 = cb[0:32, 864:992]
            upw = cb[:, 992:1376]            # upw dup'd on both halves
            b_a = [fb[:, 0:1], fb[:, 1:2]]   # per-sample-parity psa bias
            mb2p = fb[:, 2:3]
            bnd = [fb[:, 3:4], fb[:, 4:5]]   # relu bound: 0 on h-rows,
                                             # -3e38 on xd-rows, per parity

            feat_bf = cp.tile([65, 32], BF16)
            nc.vector.memset(feat_bf[:], 0.0)
            nc.vector.memset(feat_bf[64:65, :], 1.0)
            warm = cp.tile([64, 512], BF16)
            nc.vector.memset(warm[:], 0.0)
            with tc.tile_pool(name="psW", bufs=1, space="PSUM") as psW:
                psw = psW.tile([64, 512], FP32)
                for _ in range(3):
                    nc.tensor.matmul(psw[:], lhsT=warm[:, 0:64], rhs=warm[:],
                                     start=True, stop=True)
            featP = cp.tile([128, NPAIR], FP32)
            featQ = cp.tile([128, NPAIR], FP32)
            featPQ = [featP, featQ]
            zgate = cp.tile([128, 1], FP32)

            hx_tiles, pad_tiles, cw_tiles = [], [], []
            for s in range(S):
                hx = hxp.tile([128, L], BF16, name=f"hx{s}", tag="hx")
                hx_tiles.append(hx)
            for pr in range(NPAIR):
                pad = padp.tile([128, 900], BF16)
                nc.gpsimd.memset(pad[:], 0.0)
                pad_tiles.append(pad)
                cw = cwpp.tile([128, 9, 2, 64], BF16)
                nc.gpsimd.memset(cw[:], 0.0)
                cw_tiles.append(cw)

            # ================= phase A: meta-net / masks / feat =============
            # Software-pipelined: pair p+1's psa matmuls are interleaved
            # between pair p's small matmuls so the PE never drains, and the
            # ACT/DVE queues are emitted critical-path-first.
            with tc.tile_pool(name="xtp", bufs=4) as xtp, \
                 tc.tile_pool(name="psA", bufs=2, space="PSUM") as psA, \
                 tc.tile_pool(name="psB", bufs=4, space="PSUM") as psB, \
                 tc.tile_pool(name="sbA", bufs=4) as sbA, \
                 tc.tile_pool(name="smallA", bufs=4) as smA:
                xts = [None] * NPAIR
                prompts = [None] * NPAIR

                def emit_xt_dma(pr):
                    xt = xtp.tile([128, 2, KC, L], BF16, name=f"xt{pr}", tag="xt")
                    xts[pr] = xt
                    if pr == 0:
                        nc.sync.dma_start(   # first compute chunk leads
                            out=xt[:, 0, 0, :],
                            in_=xt_d[0, 0, 0].rearrange("p q -> p q"))
                        nc.sync.dma_start(out=cb[:], in_=cb_d[:])
                        for k in range(1, KC):
                            nc.sync.dma_start(
                                out=xt[:, 0, k, :],
                                in_=xt_d[0, 0, k].rearrange("p q -> p q"))
                        nc.sync.dma_start(
                            out=xt[:, 1, :, :],
                            in_=xt_d[0, 1].rearrange("k p q -> p k q"))
                        nc.sync.dma_start(out=fb[:], in_=fb_d[:])
                        nc.sync.dma_start(out=upb3[:], in_=ub_d[:])
                    else:
                        nc.sync.dma_start(
                            out=xt[:], in_=xt_d[pr].rearrange(
                                "s k p q -> p s k q"))

                def emit_psa(pr, h):
                    psa = psA.tile([128, L], FP32, tag="psa")
                    for n0, nw in NPOS:
                        for k in range(KC):
                            nc.tensor.matmul(
                                psa[:, n0:n0 + nw], lhsT=wab[h][:, k, :],
                                rhs=xts[pr][:, h, k, n0:n0 + nw],
                                start=(k == 0), stop=(k == KC - 1))
                    return psa

                def emit_hx(pr, h, psa):
                    # one fused drain per sample: (psa + bias) max bound,
                    # where bound is 0 on the h-rows (= relu) and -3e38 on
                    # the xd-rows (= identity). h-part at rows 64h, xd-part
                    # at rows 64*(1-h); phase B flips h to match.
                    nc.vector.tensor_scalar(
                        hx_tiles[2 * pr + h][:], psa[:],
                        b_a[h], bnd[h], ALU.add, ALU.max)

                # prologue: pair 0 psa + drains
                emit_xt_dma(0)
                for h in range(2):
                    psa = emit_psa(0, h)
                    emit_hx(0, h, psa)

                # All psB products split into 392-wide chunks: each chunk is
                # exactly one PSUM bank, so pool slots recycle at chunk
                # granularity and the psp(p+1) <- exp(p) coupling vanishes.
                CH = [(0, 512), (512, 272)]
                for pr in range(NPAIR):
                    nxt = pr + 1 if pr + 1 < NPAIR else None
                    prompt = sbA.tile([128, L], BF16, tag="prompt")
                    prompts[pr] = prompt
                    for n0, nw in CH:
                        psp = psB.tile([128, nw], FP32, tag="psb")
                        for h in range(2):
                            q0 = 64 * h     # h-part rows of sample 2pr+h
                            nc.tensor.matmul(
                                psp[q0:q0 + 64, :],
                                lhsT=mw2dup[q0:q0 + 64, :],
                                rhs=hx_tiles[2 * pr + h][q0:q0 + 64,
                                                         n0:n0 + nw],
                                start=True, stop=True)
                        nc.scalar.activation(prompt[:, n0:n0 + nw],
                                             psp[:], AF.Identity,
                                             bias=mb2p)

                    psa_n = [None, None]
                    if nxt is not None:
                        emit_xt_dma(nxt)
                        psa_n[0] = emit_psa(nxt, 0)

                    expt = sbA.tile([32, L], BF16, tag="expt")
                    zsum = [None, None]
                    for c, (n0, nw) in enumerate(CH):
                        psm = psB.tile([32, nw], FP32, tag="psb")
                        nc.tensor.matmul(psm[:], lhsT=mtT2,
                                         rhs=prompt[:, n0:n0 + nw],
                                         start=True, stop=True)
                        zsum[c] = smA.tile([32, 1], FP32, tag=f"z{c}",
                                           name=f"zs{pr}_{c}")
                        nc.scalar.activation(expt[:, n0:n0 + nw], psm[:],
                                             AF.Exp, accum_out=zsum[c][:])
                    invz = smA.tile([32, 1], FP32, tag="iz")
                    nc.vector.tensor_tensor(zsum[0][:], zsum[0][:],
                                            zsum[1][:], ALU.add)
                    nc.vector.reciprocal(invz[:], zsum[0][:])

                    if nxt is not None:
                        psa_n[1] = emit_psa(nxt, 1)

                    wones = sbA.tile([32, 128], BF16, tag="wones")
                    nc.vector.tensor_scalar_mul(wones[:], ones32, invz[:])
                    ftmp = sbA.tile([128, L], BF16, tag="ftmp")
                    for c, (n0, nw) in enumerate(CH):
                        pss = psB.tile([128, nw], FP32, tag="psb")
                        nc.tensor.matmul(pss[:], lhsT=wones[:],
                                         rhs=expt[:, n0:n0 + nw],
                                         start=True, stop=True)
                        nc.vector.tensor_mul(ftmp[:, n0:n0 + nw],
                                             pss[:],
                                             prompts[pr][:, n0:n0 + nw])
                        junk = sbA.tile([128, 392], BF16, tag="junk")
                        nc.scalar.activation(junk[:], ftmp[:, n0:n0 + nw],
                                             AF.Identity,
                                             accum_out=featPQ[c][:, pr:pr + 1])

                    if nxt is not None:
                        for h in range(2):
                            emit_hx(nxt, h, psa_n[h])

                    nc.vector.tensor_tensor(featP[:, pr:pr + 1],
                                            featP[:, pr:pr + 1],
                                            featQ[:, pr:pr + 1], ALU.add)
                    nc.vector.tensor_copy(feat_bf[0:64, 2 * pr:2 * pr + 1],
                                          featP[0:64, pr:pr + 1])
                    nc.vector.tensor_copy(feat_bf[0:64, 2 * pr + 1:2 * pr + 2],
                                          featP[64:128, pr:pr + 1])
                    if pr == NPAIR - 1:
                        # zero "gate": carries a dep on the last phase-A op so
                        # the scheduler can't hoist phase-B gelu (and its act
                        # table load) into phase A
                        nc.vector.tensor_scalar(
                            zgate[:], ftmp[:, 0:1], 0.0, None, ALU.mult)

            # ================= phase H: hypernet conv weights ===============
            # full-partition tiles: the strided-partition scatter reads then
            # stay inside one allocation for the access tracker
            cwalls = [cwp.tile([128, 9 * 512], BF16,
                               name=f"cwall{g}", tag=f"cwall{g}")
                      for g, (_, nb) in enumerate(GROUPS)]
            # pad-gelu for every pair only needs xd + the phase-A gate: emit
            # here so ACT runs them (and the gelu table load) during phase H
            for pr in range(NPAIR):
                pad3s = pad_tiles[pr].rearrange("p (r c) -> p r c", r=30)
                for h in range(2):
                    q0 = 64 - 64 * h    # xd rows of sample 2pr+h
                    nc.scalar.activation(
                        pad3s[q0:q0 + 64, 1:29, 1:29],
                        hx_tiles[2 * pr + h][q0:q0 + 64, :].rearrange(
                            "p (a b) -> p a b", a=28)[:],
                        AF.Gelu_apprx_sigmoid, bias=zgate[q0:q0 + 64])
            with tc.tile_pool(name="psH", bufs=6, space="PSUM") as psH:
                rot = 0
                for n9 in range(9):
                    hwc = hwp.tile([65, 2, 2048], BF16, tag="hw")
                    # Pool SWDGE queue: keeps these 9 transfers off the SP
                    # sequencer (~1.2us each there) and off the shared HWDGE
                    nc.gpsimd.dma_start(out=hwc[:], in_=hw_d[:, :, n9, :])
                    for g, (j8_0, nb) in enumerate(GROUPS):
                        psh = psH.tile([32 * nb, 512], FP32, tag="psh")
                        for a in range(nb):
                            h2, j4 = divmod(j8_0 + a, 4)
                            nc.tensor.matmul(
                                psh[32 * a:32 * a + 32, :],
                                lhsT=feat_bf[:, :],
                                rhs=hwc[:, h2, j4 * 512:(j4 + 1) * 512],
                                start=True, stop=True)
                        dst = cwalls[g][0:32 * nb, n9 * 512:(n9 + 1) * 512]
                        if rot == 0:
                            nc.vector.tensor_copy(dst, psh[:])
                        else:
                            nc.scalar.activation(dst, psh[:], AF.Copy)
                        rot = (rot + 1) % 2

            # pair-major, group-ascending inside a pair: the six scatters the
            # first conv needs come first, ordered to match drain completion.
            # Pairs 0-1 on the SP queue, pairs 2-3 on Pool SWDGE, so neither
            # queue's ~1.1-1.2us/DMA dispatch delays the first convs or the
            # output DMAs queued behind them.
            for pr in range(NPAIR):
                for g, (j8_0, nb) in enumerate(GROUPS):
                    for h in range(2):
                        s = 2 * pr + h
                        q = 1 - h       # storage half (matches xd layout)
                        eng = nc.sync if h == 0 else nc.gpsimd
                        eng.dma_start(
                            out=cw_tiles[pr][64 * q + 8 * j8_0:
                                             64 * q + 8 * (j8_0 + nb), :, q, :],
                            in_=cwalls[g][s:32 * nb:32].rearrange(
                                "p (i k o) -> p i k o", i=8, k=9))

            # ================= phase B: adapter conv + up ===================
            with tc.tile_pool(name="yap", bufs=3) as yap, \
                 tc.tile_pool(name="outp", bufs=4) as outp, \
                 tc.tile_pool(name="psC0", bufs=2, space="PSUM") as psC0, \
                 tc.tile_pool(name="psC1", bufs=2, space="PSUM") as psC1, \
                 tc.tile_pool(name="psU", bufs=2, space="PSUM") as psU:
                rot = 0
                convp = [None] * NPAIR

                def emit_conv(pr, taps):
                    pad3 = pad_tiles[pr].rearrange("p (r c) -> p r c", r=30)
                    if convp[pr] is None:
                        convp[pr] = (psC0.tile([128, 448], FP32, name=f"c0_{pr}", tag="c0"),
                                     psC1.tile([128, 336], FP32, name=f"c1_{pr}", tag="c1"))
                    ps0, ps1 = convp[pr]
                    for k9 in taps:
                        ky, kx = divmod(k9, 3)
                        lw = cw_tiles[pr][:, k9, :, :]
                        nc.tensor.matmul(
                            ps0[:], lhsT=lw,
                            rhs=pad3[:, ky:ky + 16, kx:kx + 28],
                            start=(k9 == 0), stop=(k9 == 8))
                        nc.tensor.matmul(
                            ps1[:], lhsT=lw,
                            rhs=pad3[:, ky + 16:ky + 28, kx:kx + 28],
                            start=(k9 == 0), stop=(k9 == 8))

                def emit_up(pr):
                    nonlocal rot
                    ps0, ps1 = convp[pr]
                    ya = yap.tile([128, L], BF16, tag="ya")
                    nc.scalar.activation(ya[:, 0:448], ps0[:],
                                         AF.Gelu_apprx_sigmoid)
                    nc.scalar.activation(ya[:, 448:784], ps1[:],
                                         AF.Gelu_apprx_sigmoid)
                    last = pr == NPAIR - 1
                    for q in range(2):
                        h = 1 - q       # sample parity stored in half q
                        outt = outp.tile([128, KC, L], BF16, tag="outt")
                        for j3 in range(KC):
                            # one 2-bank psum tile per (h, j3); the two
                            # matmuls split at the bank boundary, one drain
                            psu = psU.tile([128, L], FP32, tag="psu")
                            for n0, nw in NPOS:
                                nc.tensor.matmul(
                                    psu[:, n0:n0 + nw],
                                    lhsT=upw[64 * q:64 * q + 64,
                                             128 * j3:128 * (j3 + 1)],
                                    rhs=ya[64 * q:64 * q + 64, n0:n0 + nw],
                                    start=True, stop=True)
                            dst = outt[:, j3, :]
                            on_dve = (rot % 2 == 0) if last else (rot % 3 != 2)
                            if on_dve:
                                nc.vector.tensor_scalar(
                                    dst, psu[:], upb3[:, j3:j3 + 1],
                                    None, ALU.add)
                            else:
                                nc.scalar.activation(
                                    dst, psu[:], AF.Identity,
                                    bias=upb3[:, j3:j3 + 1])
                            rot += 1
                            if last and q == 0:
                                # chunked final DMA shortens the drain tail
                                nc.sync.dma_start(
                                    out=out_d[2 * pr + h, j3].rearrange(
                                        "p q -> p q"),
                                    in_=outt[:, j3, :])
                        if not (last and q == 0):
                            nc.sync.dma_start(
                                out=out_d[2 * pr + h].rearrange(
                                    "k p q -> p k q"),
                                in_=outt[:])

                # software pipeline: up(p)'s qgelu latency is covered by the
                # first taps of conv(p+1); its drain tail by the rest
                emit_conv(0, range(9))
                for pr in range(1, NPAIR):
                    emit_conv(pr, range(0, 3))
                    emit_up(pr - 1)
                    emit_conv(pr, range(3, 9))
                emit_up(NPAIR - 1)
    nc.finalize()
    return nc


def _prep(x, meta_w1, meta_b1, meta_w2, meta_b2, mask_token,
          hyper_w, hyper_b, down_w, down_b, up_w, up_b):
    f = lambda a: np.ascontiguousarray(np.asarray(a, dtype=np.float32))
    bf = lambda a: np.ascontiguousarray(np.asarray(a).astype(BF16_NP))
    x = f(x)
    xt = x.reshape(B, L, C).transpose(0, 2, 1)            # [B, C, L]
    xt = bf(xt).reshape(B, KC, 128, L).reshape(NCORES, NPAIR, 2, KC, 128, L)

    # psa weights: even parity [meta_w1 | down_w], odd parity swapped
    wA0 = np.concatenate([f(meta_w1), f(down_w)], axis=1)   # [384, 128]
    wA1 = np.concatenate([f(down_w), f(meta_w1)], axis=1)
    wab = np.stack([wA0, wA1]).reshape(2, KC, 128, 128).transpose(
        0, 2, 1, 3).reshape(2, 128, 384)                    # [par, p, (k m)]

    mtT2s = np.zeros((32, 128), np.float32)     # [m-pair, n-pair]
    mtT2s[0:16, 0:64] = f(mask_token)
    mtT2s[16:32, 64:128] = f(mask_token)
    ones32 = np.zeros((32, 128), np.float32)
    ones32[0:16, 0:64] = 1.0
    ones32[16:32, 64:128] = 1.0
    upw = f(up_w)                                # [64, 384]

    cbw = 2 * 384 + 64 + 32 + 128 + 384
    cb = np.zeros((128, cbw), np.float32)
    cb[:, 0:384] = wab[0]
    cb[:, 384:768] = wab[1]
    cb[0:64, 768:832] = f(meta_w2)
    cb[64:128, 768:832] = f(meta_w2)            # dup for base-partition match
    cb[:, 832:864] = mtT2s.T                    # [128 n-pair, 32 m-pair]
    cb[0:32, 864:992] = ones32
    cb[0:64, 992:1376] = upw
    cb[64:128, 992:1376] = upw                  # dup for base-partition match
    cb = bf(cb)

    fbm = np.zeros((128, 5), np.float32)
    fbm[0:64, 0] = f(meta_b1)
    fbm[64:128, 0] = f(down_b)
    fbm[0:64, 1] = f(down_b)
    fbm[64:128, 1] = f(meta_b1)
    fbm[0:64, 2] = f(meta_b2)
    fbm[64:128, 2] = f(meta_b2)
    fbm[0:64, 3] = 0.0                          # relu bound, parity 0
    fbm[64:128, 3] = -3.0e38
    fbm[0:64, 4] = -3.0e38                      # relu bound, parity 1
    fbm[64:128, 4] = 0.0

    upb3 = f(up_b).reshape(KC, 128).T            # [128, 3]

    # hypernet weights: columns packed (j8, i8, ky, kx, o); ones-row = hyper_b
    hw5 = f(hyper_w).reshape(META, DIM, DIM, 3, 3)       # [n, o, i, ky, kx]
    hwc = hw5.transpose(0, 2, 3, 4, 1).reshape(META, 8, 4608)  # [n, j8, (i8 k o)]
    hwc = hwc.reshape(META, 2, 4, 9, 512).transpose(0, 1, 3, 2, 4).reshape(
        META, 2, 9, 2048)
    hb5 = f(hyper_b).reshape(DIM, DIM, 3, 3)             # [o, i, ky, kx]
    hbc = hb5.transpose(1, 2, 3, 0).reshape(8, 4608)     # [j8, (i8 k o)]
    hbc = hbc.reshape(2, 4, 9, 512).transpose(0, 2, 1, 3).reshape(2, 9, 2048)
    hwe = np.concatenate([hwc, hbc[None]], axis=0)       # [65, 2, 9, 2048]
    hwe = bf(hwe)

    consts = {"cb": cb, "fb": fbm, "hw": hwe, "ub": np.ascontiguousarray(upb3)}
    in_maps = []
    for c in range(NCORES):
        m = dict(consts)
        m["xt"] = np.ascontiguousarray(xt[c])
        in_maps.append(m)
    return in_maps


def _run(in_maps, **kw):
    if "nc" not in _CACHE:
        _CACHE["nc"] = _build_nc()
    return run_bass_kernel_spmd(_CACHE["nc"], in_maps, list(range(NCORES)), **kw)


def kernel(x, meta_w1, meta_b1, meta_w2, meta_b2, mask_token,
           hyper_w, hyper_b, down_w, down_b, up_w, up_b, H, W):
    assert int(H) == HH and int(W) == WW
    in_maps = _prep(x, meta_w1, meta_b1, meta_w2, meta_b2, mask_token,
                    hyper_w, hyper_b, down_w, down_b, up_w, up_b)
    res = _run(in_maps)
    outs = []
    for c in range(NCORES):
        o = np.asarray(res.results[c]["out"]).astype(np.float32)
        # [S, KC, 128, L] -> [S, C, L] -> [S, L, C]
        o = o.reshape(S, C, L).transpose(0, 2, 1)
        outs.append(o)
    out = np.concatenate(outs, axis=0)
    return np.ascontiguousarray(out.reshape(B, L, C)).astype(np.float32)
